# revision 1
# baseline (speedup 1.0000x reference)
"""Causal self-attention block (LN -> QKV -> causal attention -> out-proj)
on 8 Trainium2 NeuronCores.

Sharding: core = 2*batch + head_group. Each core handles one batch element
(S=2048 tokens) and 8 of the 16 heads (tensor-parallel split of w_qkv along
the head axis and w_out along its input dim). The two partial outputs per
batch are summed on the host (the all-reduce of the sharding hint).

Device kernel layout strategy (per core):
  - LayerNorm in natural layout [s, d], then PE-transpose to xnT [d, s]
    (contraction dim must sit on partitions for matmuls).
  - QKV projection computes q^T/k^T in [head_dim, s] layout directly and V in
    natural [s, head_dim] layout, so causal attention needs no further
    transposes: scores are computed transposed, ST[k, q] = k . q, softmax'd
    along the partition-free axis via exp + a ones-column appended to V
    (the PV matmul then yields both y^T and the softmax row-sums).
  - ln_scale/ln_bias/b_qkv/softmax-scale/b_out are all folded into the
    weights on the host; matmuls run as float32r (FP22, full PE rate).
"""

import os

# the device path runs through jax's axon PJRT plugin; make sure a
# pre-set JAX_PLATFORMS doesn't hide it (unset = all plugins load)
_jp = os.environ.get("JAX_PLATFORMS")
if _jp and "axon" not in _jp:
    os.environ["JAX_PLATFORMS"] = f"axon,{_jp}"

import numpy as np

import concourse.bass as bass
import concourse.mybir as mybir
import concourse.tile as tile
from concourse import bacc
from concourse.bass_utils import run_bass_kernel_spmd
from concourse.masks import make_identity

B, S, D, H, HD = 4, 2048, 1024, 16, 64
HL = H // 2          # heads per core (local)
NCH = D // 128       # 8 contraction chunks
NSB = S // 128       # 16 s-blocks
NQS = S // 512       # 4 q-superblocks
NEG = -1.0e38
LN_EPS = 1e-6

f32 = mybir.dt.float32
f32r = mybir.dt.float32r

_cache = {}


def build_program():
    nc = bacc.Bacc()

    x_d = nc.declare_dram_parameter("x", [S, D], f32, isOutput=False)
    wqk_d = nc.declare_dram_parameter("wqk", [NCH, 128, 1024], f32r, isOutput=False)
    wv_d = nc.declare_dram_parameter("wv", [NCH, 128, 512], f32r, isOutput=False)
    bqk_d = nc.declare_dram_parameter("bqk", [128, 2, 4], f32, isOutput=False)
    bv1_d = nc.declare_dram_parameter("bv1", [1, 512], f32r, isOutput=False)
    vones_d = nc.declare_dram_parameter("vones", [1, 128], f32r, isOutput=False)
    wout_d = nc.declare_dram_parameter("wout", [4, 128, 1024], f32r, isOutput=False)
    out_d = nc.declare_dram_parameter("out", [S, D], f32, isOutput=True)

    with tile.TileContext(nc, pool_alloc_mode="queue") as tc:
        with (
            tc.tile_pool(name="singles", bufs=1) as singles,
            tc.tile_pool(name="qkT", bufs=1) as qkTp,
            tc.tile_pool(name="vpool", bufs=1) as vpool,
            tc.tile_pool(name="pscm", bufs=1, space="PSUM") as pscm,
        ):
            # ---- constants ----
            ident = singles.tile([128, 128], f32)
            make_identity(nc, ident)
            identb = singles.tile([128, 128], mybir.dt.bfloat16)
            make_identity(nc, identb)
            maskTb = singles.tile([128, 128], mybir.dt.bfloat16)
            nc.gpsimd.memset(maskTb, 0.0)
            nc.gpsimd.affine_select(
                out=maskTb, in_=maskTb,
                compare_op=mybir.AluOpType.is_ge,
                fill=NEG, base=0,
                pattern=[[1, 128]], channel_multiplier=-1,
            )
            eps_t = singles.tile([128, 1], f32)
            nc.vector.memset(eps_t, LN_EPS)
            bqk_t = singles.tile([128, 2, 4], f32)
            nc.sync.dma_start(out=bqk_t, in_=bqk_d[:, :, :])
            bv1_t = singles.tile([1, 512], f32r)
            nc.sync.dma_start(out=bv1_t, in_=bv1_d[:, :])
            vones_t = singles.tile([1, 128], f32r)
            nc.sync.dma_start(out=vones_t, in_=vones_d[:, :])

            # ---- persistent activations ----
            qT = qkTp.tile([128, 4, S], f32r)   # [pair-row, pair, s]
            kT = qkTp.tile([128, 4, S], f32r)
            # V'' [s-row, s-block, head, 65] (col 64 = ones)
            vpp = vpool.tile([128, NSB, HL, HD + 1], f32r)
            nc.gpsimd.memset(vpp[:, :, :, HD : HD + 1].bitcast(f32), 1.0)

            # ================= Phase A: LayerNorm + transpose =================
            with tc.tile_pool(name="xnT", bufs=1) as xnTp:
                xnT = xnTp.tile([128, NCH, S], f32r)
                with (
                    tc.tile_pool(name="atmp", bufs=5) as atmp,
                    tc.tile_pool(name="astat", bufs=8) as astat,
                ):
                    for i in range(NSB):
                        x_t = atmp.tile([128, D], f32, tag="x")
                        nc.sync.dma_start(out=x_t, in_=x_d[i * 128 : (i + 1) * 128, :])
                        stats = astat.tile([128, 2, 6], f32, tag="stats")
                        nc.vector.bn_stats(out=stats[:, 0, :], in_=x_t[:, 0:512])
                        nc.vector.bn_stats(out=stats[:, 1, :], in_=x_t[:, 512:1024])
                        mv = astat.tile([128, 2], f32, tag="mv")
                        nc.vector.bn_aggr(out=mv, in_=stats)
                        std_t = astat.tile([128, 1], f32, tag="std")
                        nc.scalar.activation(
                            out=std_t, in_=mv[:, 1:2],
                            func=mybir.ActivationFunctionType.Sqrt,
                            bias=eps_t, scale=1.0,
                        )
                        rstd_t = astat.tile([128, 1], f32, tag="rstd")
                        nc.vector.reciprocal(out=rstd_t, in_=std_t)
                        xn_t = atmp.tile([128, D], f32, tag="xn")
                        nc.vector.tensor_scalar(
                            out=xn_t, in0=x_t,
                            scalar1=mv[:, 0:1], scalar2=rstd_t,
                            op0=mybir.AluOpType.subtract, op1=mybir.AluOpType.mult,
                        )
                        for c4 in range(0, NCH, 4):
                            pst = pscm.tile([128, 4, 128], f32, tag="yt", bufs=4)
                            for c in range(c4, c4 + 4):
                                nc.tensor.transpose(
                                    pst[:, c - c4, :],
                                    xn_t[:, c * 128 : (c + 1) * 128],
                                    ident,
                                )
                            nc.scalar.activation(
                                out=xnT[:, c4 : c4 + 4, i * 128 : (i + 1) * 128],
                                in_=pst,
                                func=mybir.ActivationFunctionType.Copy,
                            )

                # ================= Phase B: QKV projection =================
                with (
                    tc.tile_pool(name="wqk", bufs=2) as wqkp,
                    tc.tile_pool(name="wvp", bufs=1) as wvp,
                ):
                    def emit_qk(t, p):
                        fb = t * 4 + p
                        w_t = wqkp.tile([128, NCH, 128], f32r, tag="wqk",
                                        name=f"wqk_{t}_{p}")
                        nc.sync.dma_start(
                            out=w_t,
                            in_=wqk_d[:, :, fb * 128 : (fb + 1) * 128].rearrange(
                                "c d f -> d c f"
                            ),
                        )
                        dest = qT if t == 0 else kT
                        for sb in range(NQS):
                            ps = pscm.tile([128, 512], f32, tag="st", bufs=2,
                                           name=f"psqk_{t}_{p}_{sb}")
                            for c in range(NCH):
                                nc.tensor.matmul(
                                    ps,
                                    w_t[:, c, :],
                                    xnT[:, c, sb * 512 : (sb + 1) * 512],
                                    start=(c == 0),
                                    stop=(c == NCH - 1),
                                )
                            nc.vector.tensor_scalar_add(
                                out=dest[:, p, sb * 512 : (sb + 1) * 512],
                                in0=ps,
                                scalar1=bqk_t[:, t, p : p + 1],
                            )

                    def emit_v():
                        wv_t = wvp.tile([128, NCH, 512], f32r)
                        for c in range(NCH):
                            nc.sync.dma_start(out=wv_t[:, c, :], in_=wv_d[c, :, :])
                        for i in range(NSB):
                            psv = pscm.tile([128, 512], f32, tag="st", bufs=2,
                                            name=f"psv_{i}")
                            for c in range(NCH):
                                nc.tensor.matmul(
                                    psv,
                                    xnT[:, c, i * 128 : (i + 1) * 128],
                                    wv_t[:, c, :],
                                    start=(c == 0),
                                    stop=False,
                                )
                            # += ones[s] x bv  (rank-1 bias update)
                            nc.tensor.matmul(
                                psv, vones_t, bv1_t, start=False, stop=True,
                            )
                            nc.vector.tensor_copy(
                                vpp[:, i, :, 0:HD],
                                psv.rearrange("p (h v) -> p h v", v=HD),
                            )

                    # pair 0 first, then V, so attention on heads 0/1 can
                    # start while the rest of the projection still runs
                    emit_qk(0, 0)
                    emit_qk(1, 0)
                    emit_v()
                    for p in range(1, 4):
                        emit_qk(0, p)
                        emit_qk(1, p)

            # ================= Phase C: causal attention =================
            with tc.tile_pool(name="ytall", bufs=1) as ytallp:
                ytall = ytallp.tile([128, 4, S], f32r)  # [pair-row, pair, s]
                with (
                    tc.tile_pool(name="ptp", bufs=6) as ptp,
                    tc.tile_pool(name="ctmp", bufs=4) as ctmp,
                    tc.tile_pool(name="dscr", bufs=8, space="DRAM") as dscr,
                    tc.tile_pool(name="woutp", bufs=1) as woutp,
                    tc.tile_pool(name="ypool", bufs=3) as ypool,
                ):
                    wout_t = woutp.tile([128, 4, 1024], f32r)
                    for c in range(4):
                        nc.sync.dma_start(out=wout_t[:, c, :], in_=wout_d[c, :, :])
                    def emit_outproj(i):
                        y_t = ypool.tile([128, 1024], f32, tag="y",
                                         name=f"y_{i}")
                        for nh in range(2):
                            # alternate tags: the yt slots are idle during
                            # the output projection, use them for depth
                            pso = pscm.tile([128, 512], f32,
                                            tag=("st" if nh == 0 else "yt"),
                                            bufs=(2 if nh == 0 else 4),
                                            name=f"pso_{i}_{nh}")
                            for c in range(4):
                                nc.tensor.matmul(
                                    pso,
                                    ytall[:, c, i * 128 : (i + 1) * 128],
                                    wout_t[:, c, nh * 512 : (nh + 1) * 512],
                                    start=(c == 0),
                                    stop=(c == 3),
                                )
                            nc.vector.tensor_copy(
                                y_t[:, nh * 512 : (nh + 1) * 512], pso
                            )
                        nc.sync.dma_start(
                            out=out_d[i * 128 : (i + 1) * 128, :], in_=y_t
                        )

                    for sb in range(NQS):
                        for p in range(4):
                            # the pair's two heads (PE rows 0:64 / 64:128)
                            # run as adjacent matmuls -> concurrent row-groups
                            q0 = sb * 512
                            jmax = 4 * sb + 3
                            yts = [
                                pscm.tile([HD + 1, 512], f32, tag="yt",
                                          bufs=4, name=f"yt_{2 * p + hf}_{sb}")
                                for hf in range(2)
                            ]
                            for j in range(jmax + 1):
                                r = max(0, j - 4 * sb)
                                diag = j >= 4 * sb
                                L = 512 - 128 * r
                                st = pscm.tile([128, 1024], f32, tag="st",
                                               bufs=2, name=f"st_{p}_{sb}_{j}")
                                pt = ptp.tile([128, 1024], f32r, tag="pt")
                                for hf in range(2):
                                    rows = slice(hf * HD, (hf + 1) * HD)
                                    # hf0 packs left in bank 0; hf1 must stay
                                    # bank-aligned at 512 (matmul outputs
                                    # cannot cross a PSUM bank boundary)
                                    lo = hf * 512
                                    nc.tensor.matmul(
                                        st[:, lo : lo + L],
                                        kT[rows, p, j * 128 : (j + 1) * 128],
                                        qT[rows, p, q0 + r * 128 : q0 + 512],
                                        start=True, stop=not diag,
                                    )
                                if diag:
                                    # causal mask folded in on the PE:
                                    # st[diag] += I.T @ maskT
                                    for hf in range(2):
                                        nc.tensor.matmul(
                                            st[:, hf * 512 : hf * 512 + 128],
                                            identb,
                                            maskTb,
                                            start=False, stop=True,
                                        )
                                # one wide exp across both heads (for r>0 the
                                # [L:512) strip is unread garbage)
                                nc.scalar.activation(
                                    out=pt[:, 0 : 512 + L],
                                    in_=st[:, 0 : 512 + L],
                                    func=mybir.ActivationFunctionType.Exp,
                                )
                                for hf in range(2):
                                    nc.tensor.matmul(
                                        yts[hf][:, r * 128 : 512],
                                        vpp[:, j, 2 * p + hf, :],
                                        pt[:, hf * 512 : hf * 512 + L],
                                        start=(j == 0),
                                        stop=(j == jmax),
                                    )
                            # per-superblock softmax normalization epilogue
                            for hf in range(2):
                                rows = slice(hf * HD, (hf + 1) * HD)
                                yt = yts[hf]
                                ssum = ctmp.tile([1, 512], f32, tag="ssum")
                                nc.vector.tensor_copy(ssum, yt[HD : HD + 1, :])
                                dsum = dscr.tile([512], f32, tag="dsum")
                                nc.sync.dma_start(out=dsum, in_=ssum)
                                sums4 = ctmp.tile([4, 128], f32, tag="sums4")
                                nc.sync.dma_start(
                                    out=sums4,
                                    in_=dsum.rearrange("(a b) -> a b", b=128),
                                )
                                sinv4 = ctmp.tile([4, 128], f32, tag="sinv4")
                                nc.vector.reciprocal(out=sinv4, in_=sums4)
                                dsinv = dscr.tile([512], f32, tag="dsinv")
                                nc.sync.dma_start(
                                    out=dsinv.rearrange("(a b) -> a b", b=128),
                                    in_=sinv4,
                                )
                                src = dsinv[:]
                                bcast = bass.AP(
                                    tensor=src.tensor,
                                    offset=src.offset,
                                    ap=[[0, HD]] + list(src.ap),
                                )
                                binv = ctmp.tile([HD, 512], f32, tag="binv")
                                nc.sync.dma_start(out=binv, in_=bcast)
                                nc.vector.tensor_mul(
                                    out=ytall[rows, p, q0 : q0 + 512],
                                    in0=yt[0:HD, :],
                                    in1=binv,
                                )

                    for i in range(NSB):
                        emit_outproj(i)

    nc.finalize()
    return nc


def _prep_core_inputs(x, ln_scale, ln_bias, w_qkv, b_qkv, w_out):
    """Host-side folding + per-core input maps."""
    scale = np.float32(HD ** -0.5)
    # qkv = xn@W + b_qkv, xn = z*ln_scale + ln_bias  =>  z @ (ln_scale*W) + (ln_bias@W + b_qkv)
    b_eff = b_qkv + np.einsum(
        "d,dhf->hf", ln_bias.astype(np.float64), w_qkv.astype(np.float64)
    ).astype(np.float32)
    w_eff = ln_scale[:, None, None] * w_qkv
    wq = w_eff[:, :, 0:64] * scale
    wk = w_eff[:, :, 64:128]
    wv = w_eff[:, :, 128:192]
    bq = b_eff[:, 0:64] * scale
    bk = b_eff[:, 64:128]
    bv = b_eff[:, 128:192]

    in_maps = []
    for core in range(8):
        b, g = core // 2, core % 2
        hsel = slice(g * HL, (g + 1) * HL)
        # [D, 4 pairs, 128] with head 2p in rows 0:64, head 2p+1 in 64:128
        qp = wq[:, hsel].reshape(D, 4, 128)
        kp = wk[:, hsel].reshape(D, 4, 128)
        wqk = np.concatenate(
            [qp.reshape(D, 512), kp.reshape(D, 512)], axis=1
        ).reshape(NCH, 128, 1024)
        wv_g = np.ascontiguousarray(wv[:, hsel].reshape(D, 512)).reshape(
            NCH, 128, 512
        )
        bq_p = bq[hsel].reshape(4, 128)
        bk_p = bk[hsel].reshape(4, 128)
        bqk = np.ascontiguousarray(
            np.stack([bq_p, bk_p], axis=0).transpose(2, 0, 1)
        )
        bv1 = np.ascontiguousarray(bv[hsel].reshape(1, 512))
        wout = np.ascontiguousarray(
            w_out[g * 512 : (g + 1) * 512, :].reshape(4, 128, 1024)
        )
        in_maps.append(
            {
                "x": np.ascontiguousarray(x[b]),
                "wqk": np.ascontiguousarray(wqk),
                "wv": wv_g,
                "bqk": bqk,
                "bv1": bv1,
                "vones": np.ones((1, 128), np.float32),
                "wout": wout,
            }
        )
    return in_maps


def kernel(x, mask, ln_scale, ln_bias, w_qkv, b_qkv, w_out, b_out, **run_kwargs):
    x = np.asarray(x, np.float32)
    ln_scale = np.asarray(ln_scale, np.float32)
    ln_bias = np.asarray(ln_bias, np.float32)
    w_qkv = np.asarray(w_qkv, np.float32)
    b_qkv = np.asarray(b_qkv, np.float32)
    w_out = np.asarray(w_out, np.float32)
    b_out = np.asarray(b_out, np.float32)
    if "nc" not in _cache:
        _cache["nc"] = build_program()
    nc = _cache["nc"]
    in_maps = _prep_core_inputs(x, ln_scale, ln_bias, w_qkv, b_qkv, w_out)
    res = run_bass_kernel_spmd(nc, in_maps, list(range(8)), **run_kwargs)
    _cache["last_result"] = res
    out = np.empty((B, S, D), np.float32)
    for b in range(B):
        out[b] = res.results[2 * b]["out"] + res.results[2 * b + 1]["out"]
    out += np.asarray(b_out)[None, None, :]
    return out



# revision 56
# speedup vs baseline: 1.2091x; 1.2091x over previous
"""Causal self-attention block (LN -> QKV -> causal attention -> out-proj)
on 8 Trainium2 NeuronCores.

Sharding: core = 2*batch + head_group. Each core handles one batch element
(S=2048 tokens) and 8 of the 16 heads (tensor-parallel split of w_qkv along
the head axis and w_out along its input dim). The two partial outputs per
batch are summed on the host (the all-reduce of the sharding hint).

v2 kernel layout strategy (per core), all matmuls in bf16 (1 cycle/row at
any moving width, validated ~2e-3 end-to-end rel err on host):
  - LayerNorm in natural layout [s, d] (DVE stats), then PE-transpose the
    bf16 xn to xnT [d, s] per 512-token superblock.
  - QKV computes qT/kT in [head_dim, s] layout and V in natural [s, hd].
  - Scores are computed transposed, ST[k, q] = k.q (causal mask folded in
    on the PE via a NEG upper-tri addend), exp on ACT writes bf16 PT.
  - PV runs in natural layout: y[q, hd] accumulates over k-blocks with PT
    slices as the stationary operand and V (+ ones column) moving; the
    ones column yields softmax row-sums per-partition, so normalization is
    a per-partition reciprocal + scalar multiply (no cross-partition
    broadcast, no DMA round-trips).
  - Normalized y is PE-transposed back to yT [d_local, s] for the output
    projection.
  - Persistent tensors are split into per-block tiles so phases overlap
    through slice-exact dependencies.
"""

import os

_jp = os.environ.get("JAX_PLATFORMS")
if _jp and "axon" not in _jp:
    os.environ["JAX_PLATFORMS"] = f"axon,{_jp}"

import ml_dtypes
import numpy as np

import concourse.bass as bass
import concourse.mybir as mybir
import concourse.tile as tile
from concourse import bacc
from concourse.bass_utils import run_bass_kernel_spmd
from concourse.masks import make_identity

B, S, D, H, HD = 4, 2048, 1024, 16, 64
HL = H // 2          # heads per core (local)
NCH = D // 128       # 8 contraction chunks
NSB = S // 128       # 16 s-blocks
NQS = S // 512       # 4 superblocks
NEG = -1.0e38
LN_EPS = 1e-6

f32 = mybir.dt.float32
bf16 = mybir.dt.bfloat16
npbf16 = ml_dtypes.bfloat16

_cache = {}

# bisection knobs (full kernel: 4, False, False, False)
N_WAVES = 4
NO_ATTN = False
NO_EPI = False
NO_OUTPROJ = False


def build_program():
    nc = bacc.Bacc()

    x_d = nc.declare_dram_parameter("x", [S, D], bf16, isOutput=False)
    wqk_d = nc.declare_dram_parameter("wqk", [128, 8, NCH, 128], bf16, isOutput=False)
    wv_d = nc.declare_dram_parameter("wv", [128, NCH, 512], bf16, isOutput=False)
    bqk_d = nc.declare_dram_parameter("bqk", [128, 2, 4], f32, isOutput=False)
    bv1_d = nc.declare_dram_parameter("bv1", [1, 512], bf16, isOutput=False)
    vones_d = nc.declare_dram_parameter("vones", [1, 128], bf16, isOutput=False)
    wout_d = nc.declare_dram_parameter("wout", [128, 4, 1024], bf16, isOutput=False)
    out_d = nc.declare_dram_parameter("out", [S, D], f32, isOutput=True)

    with tile.TileContext(nc, pool_alloc_mode="queue") as tc:
        with (
            tc.tile_pool(name="singles", bufs=1) as singles,
            tc.tile_pool(name="persist", bufs=1) as persist,
            tc.tile_pool(name="pscm", bufs=1, space="PSUM") as pscm,
            tc.tile_pool(name="atmp", bufs=4) as atmp,
            tc.tile_pool(name="astat", bufs=8) as astat,
            tc.tile_pool(name="ptp", bufs=3) as ptp,
            tc.tile_pool(name="cpool", bufs=2) as cpool,
            tc.tile_pool(name="ypool", bufs=3) as ypool,
        ):
            # ---- constants ----
            identb = singles.tile([128, 128], bf16)
            make_identity(nc, identb)
            maskTb = singles.tile([128, 128], bf16)
            nc.gpsimd.memset(maskTb, 0.0)
            nc.gpsimd.affine_select(
                out=maskTb, in_=maskTb,
                compare_op=mybir.AluOpType.is_ge,
                fill=NEG, base=0,
                pattern=[[1, 128]], channel_multiplier=-1,
            )
            eps_t = singles.tile([128, 1], f32)
            nc.vector.memset(eps_t, LN_EPS)

            # ---- persistent per-block tiles ----
            xnT_t = [persist.tile([128, NCH, 512], bf16, tag=f"xnT{s4}",
                                  name=f"xnT{s4}") for s4 in range(NQS)]
            qT_t = {(p, s4): persist.tile([128, 512], bf16, tag=f"qT{p}_{s4}",
                                          name=f"qT{p}_{s4}")
                    for p in range(4) for s4 in range(NQS)}
            kT_t = {(p, s4): persist.tile([128, 512], bf16, tag=f"kT{p}_{s4}",
                                          name=f"kT{p}_{s4}")
                    for p in range(4) for s4 in range(NQS)}
            vpp_t = [persist.tile([128, HL, HD + 1], bf16, tag=f"vpp{j}",
                                  name=f"vpp{j}") for j in range(NSB)]
            yT_t = {}   # filled by attention epilogues with yTq slices
            for j in range(NSB):
                nc.gpsimd.memset(vpp_t[j][:, :, HD : HD + 1], 1.0)

            # ---- weights ----
            wqk_all = persist.tile([128, 8, NCH, 128], bf16, tag="wqk")
            wqk_w = [wqk_all[:, fb] for fb in range(8)]
            wv_w = persist.tile([128, NCH, 512], bf16, tag="wv")
            wout_w = persist.tile([128, 4, 1024], bf16, tag="wout")

            # ================= Phase A: LayerNorm + transpose =================
            x_tiles = {}

            def emit_x_dma(i):
                x_t = atmp.tile([128, D], bf16, tag="x", bufs=12, name=f"x_{i}")
                nc.sync.dma_start(out=x_t, in_=x_d[i * 128 : (i + 1) * 128, :])
                x_tiles[i] = x_t

            def emit_ln_block(i):
                x_t = x_tiles.pop(i)
                stats = astat.tile([128, 2, 6], f32, tag="stats")
                nc.vector.bn_stats(out=stats[:, 0, :], in_=x_t[:, 0:512])
                nc.vector.bn_stats(out=stats[:, 1, :], in_=x_t[:, 512:1024])
                mv = astat.tile([128, 2], f32, tag="mv")
                nc.vector.bn_aggr(out=mv, in_=stats)
                std_t = astat.tile([128, 1], f32, tag="std")
                nc.scalar.activation(
                    out=std_t, in_=mv[:, 1:2],
                    func=mybir.ActivationFunctionType.Sqrt,
                    bias=eps_t, scale=1.0,
                )
                rstd_t = astat.tile([128, 1], f32, tag="rstd")
                nc.vector.reciprocal(out=rstd_t, in_=std_t)
                xn_t = atmp.tile([128, D], bf16, tag="xn", bufs=3, name=f"xn_{i}")
                if i % 4 == 1:
                    # ACT path: xn = (x - mu)*rstd = x*rstd + (-mu*rstd)
                    nb = astat.tile([128, 1], f32, tag="nb")
                    nc.vector.tensor_scalar(
                        out=nb, in0=mv[:, 0:1],
                        scalar1=rstd_t, scalar2=-1.0,
                        op0=mybir.AluOpType.mult, op1=mybir.AluOpType.mult,
                    )
                    nc.scalar.activation(
                        out=xn_t, in_=x_t,
                        func=mybir.ActivationFunctionType.Identity,
                        bias=nb, scale=rstd_t,
                    )
                else:
                    norm_eng = nc.vector if i % 4 == 3 else nc.gpsimd
                    norm_eng.tensor_scalar(
                        out=xn_t, in0=x_t,
                        scalar1=mv[:, 0:1], scalar2=rstd_t,
                        op0=mybir.AluOpType.subtract, op1=mybir.AluOpType.mult,
                    )
                s4, ib = i // 4, i % 4
                pst = pscm.tile([128, NCH, 128], bf16, tag="aux", bufs=1,
                                name=f"pst_{i}")
                for c in range(NCH):
                    nc.tensor.transpose(
                        pst[:, c, :],
                        xn_t[:, c * 128 : (c + 1) * 128],
                        identb,
                    )
                dst = xnT_t[s4][:, :, ib * 128 : (ib + 1) * 128]
                if i % 2 == 0:
                    nc.scalar.activation(
                        out=dst, in_=pst,
                        func=mybir.ActivationFunctionType.Copy,
                    )
                else:
                    nc.vector.tensor_copy(dst, pst)

            # ================= Phase B: QKV projection =================
            def emit_qk(t, p, s4):
                fb = t * 4 + p
                dest = qT_t if t == 0 else kT_t
                ps = pscm.tile([128, 512], f32,
                               tag=("qkv" if fb % 2 == 0 else "aux"), bufs=1,
                               name=f"psqk_{t}_{p}_{s4}")
                # per-128-block columns so each starts as soon as its LN
                # block's transpose lands
                for ib in range(4):
                    cols = slice(ib * 128, (ib + 1) * 128)
                    for c in range(NCH):
                        nc.tensor.matmul(
                            ps[:, cols],
                            wqk_w[fb][:, c, :],
                            xnT_t[s4][:, c, cols],
                            start=(c == 0),
                            stop=(c == NCH - 1),
                        )
                nc.vector.tensor_scalar_add(
                    out=dest[(p, s4)], in0=ps,
                    scalar1=bqk_t[:, t, p : p + 1],
                )

            def emit_v(i):
                s4, ib = i // 4, i % 4
                psv = pscm.tile([128, 512], f32,
                                tag=("qkv" if i % 2 == 0 else "aux"), bufs=1,
                                name=f"psv_{i}")
                for c in range(NCH):
                    nc.tensor.matmul(
                        psv,
                        xnT_t[s4][:, c, ib * 128 : (ib + 1) * 128],
                        wv_w[:, c, :],
                        start=(c == 0),
                        stop=False,
                    )
                nc.tensor.matmul(psv, vones_t, bv1_t, start=False, stop=True)
                nc.vector.tensor_copy(
                    vpp_t[i][:, :, 0:HD],
                    psv.rearrange("p (h v) -> p h v", v=HD),
                )

            # ================= Phase C: causal attention =================
            def emit_attn(s4, p):
                q0 = s4 * 512
                jmax = 4 * s4 + 3
                ya = pscm.tile([128, 2, 4, 128], f32, tag="ya", bufs=1,
                               name=f"ya_{s4}_{p}")
                yacc = [ya[:, 0], ya[:, 1]]
                pts = {}

                def emit_scores(j):
                    r = max(0, j - 4 * s4)
                    diag = j >= 4 * s4
                    L = 512 - 128 * r
                    hb = 512
                    st = pscm.tile([128, 1024], f32, tag="st", bufs=2,
                                   name=f"st_{s4}_{p}_{j}")
                    kt = kT_t[(p, j // 4)]
                    # bank-aligned matmul outputs only (mid-bank column
                    # offsets fault the device); mask accumulates after
                    for hf in range(2):
                        rows = slice(hf * HD, (hf + 1) * HD)
                        nc.tensor.matmul(
                            st[:, hf * 512 : hf * 512 + L],
                            kt[rows, (j % 4) * 128 : (j % 4 + 1) * 128],
                            qT_t[(p, s4)][rows, r * 128 : 512],
                            start=True, stop=not diag,
                        )
                    if diag:
                        for hf in range(2):
                            nc.tensor.matmul(
                                st[:, hf * 512 : hf * 512 + 128],
                                identb, maskTb,
                                start=False, stop=True,
                            )
                    pt = ptp.tile([128, 1024], bf16, tag="pt", bufs=5,
                                  name=f"pt_{s4}_{p}_{j}")
                    if L < 512:
                        # [L, 512) was never written; exp the two written
                        # ranges separately
                        for hf in range(2):
                            nc.scalar.activation(
                                out=pt[:, hf * 512 : hf * 512 + L],
                                in_=st[:, hf * 512 : hf * 512 + L],
                                func=mybir.ActivationFunctionType.Exp,
                            )
                    else:
                        nc.scalar.activation(
                            out=pt[:, 0 : 512 + L], in_=st[:, 0 : 512 + L],
                            func=mybir.ActivationFunctionType.Exp,
                        )
                    pts[j] = (pt, r, hb)

                def emit_pv(j):
                    # PSUM has one accumulation group per 2KB bank: start
                    # only on the first matmul into each hf's bank (zeroes
                    # whole-bank has_written); later first-touches of other
                    # qb columns overwrite via per-element has_written.
                    pt, r, hb = pts.pop(j)
                    for hf in range(2):
                        head = 2 * p + hf
                        for qb in range(r, 4):
                            base = hf * hb + (qb - r) * 128
                            nc.tensor.matmul(
                                yacc[hf][:, qb, 0 : HD + 1],
                                pt[:, base : base + 128],
                                vpp_t[j][:, head, :],
                                start=(j == 0 and qb == 0),
                                stop=(j == jmax and qb == 3),
                            )

                emit_scores(0)
                for j in range(1, jmax + 1):
                    emit_scores(j)
                    emit_pv(j - 1)
                emit_pv(jmax)
                if NO_EPI:
                    return

                # epilogue: per-partition softmax normalization
                ynat = cpool.tile([128, 4, 128], bf16, tag="ynat",
                                  name=f"ynat_{s4}_{p}")
                rc = cpool.tile([128, 2, 4, 1], f32, tag="rc",
                                name=f"rc_{s4}_{p}")
                nc.vector.reciprocal(out=rc, in_=ya[:, :, :, HD : HD + 1])
                yraw = cpool.tile([128, 2, 4, HD], f32, tag="yraw",
                                  name=f"yraw_{s4}_{p}")
                nc.vector.tensor_copy(yraw, ya[:, :, :, 0:HD])
                for qb in range(4):
                    for hf in range(2):
                        nc.gpsimd.tensor_scalar_mul(
                            out=ynat[:, qb, hf * HD : (hf + 1) * HD],
                            in0=yraw[:, hf, qb, :],
                            scalar1=rc[:, hf, qb, :],
                        )
                ytp = pscm.tile([128, 4, 128], bf16, tag="aux", bufs=1,
                                name=f"ytp_{s4}_{p}")
                for qb in range(4):
                    nc.tensor.transpose(ytp[:, qb, :], ynat[:, qb, :], identb)
                yTq = persist.tile([128, 4, 128], bf16, tag=f"yTq_{s4}_{p}",
                                   name=f"yTq_{s4}_{p}")
                nc.vector.tensor_copy(yTq, ytp)
                for qb in range(4):
                    yT_t[(4 * s4 + qb, p)] = yTq[:, qb, :]

            # ================= out-projection =================
            def emit_outproj(i):
                y_t = ypool.tile([128, 1024], f32, tag="y", name=f"y_{i}")
                for nh in range(2):
                    pso = pscm.tile([128, 512], f32,
                                    tag=("aux" if nh == 0 else "qkv"), bufs=1,
                                    name=f"pso_{i}_{nh}")
                    for c in range(4):
                        nc.tensor.matmul(
                            pso,
                            yT_t[(i, c)],
                            wout_w[:, c, nh * 512 : (nh + 1) * 512],
                            start=(c == 0),
                            stop=(c == 3),
                        )
                    nc.vector.tensor_copy(y_t[:, nh * 512 : (nh + 1) * 512], pso)
                nc.sync.dma_start(out=out_d[i * 128 : (i + 1) * 128, :], in_=y_t)

            # ---- emission schedule: software-pipelined waves ----
            # LN runs one wave ahead of QKV, which runs with attention of
            # the prior wave; outproj of wave w-1 interleaves into wave w.
            for i in range(4):
                emit_x_dma(i)
            for fb in (0, 4):
                nc.sync.dma_start(out=wqk_all[:, fb], in_=wqk_d[:, fb])
            for i in range(4, 8):
                emit_x_dma(i)
            for fb in (1, 5):
                nc.sync.dma_start(out=wqk_all[:, fb], in_=wqk_d[:, fb])
            nc.sync.dma_start(out=wv_w, in_=wv_d[:, :, :])
            for i in range(8, 12):
                emit_x_dma(i)
            for fb in (2, 6, 3, 7):
                nc.sync.dma_start(out=wqk_all[:, fb], in_=wqk_d[:, fb])
            nc.sync.dma_start(out=wout_w, in_=wout_d[:, :, :])
            bqk_t = singles.tile([128, 2, 4], f32)
            nc.sync.dma_start(out=bqk_t, in_=bqk_d[:, :, :])
            bv1_t = singles.tile([1, 512], bf16)
            nc.sync.dma_start(out=bv1_t, in_=bv1_d[:, :])
            vones_t = singles.tile([1, 128], bf16)
            nc.sync.dma_start(out=vones_t, in_=vones_d[:, :])
            for i in range(0, min(8, 4 * N_WAVES)):
                emit_ln_block(i)        # waves 0+1
            for s4 in range(N_WAVES):
                for p in range(4):
                    emit_qk(0, p, s4)
                    emit_qk(1, p, s4)
                for i in range(4 * s4, 4 * s4 + 4):
                    emit_v(i)
                if s4 == 0:
                    for i in range(12, NSB):
                        emit_x_dma(i)
                if s4 < 2 and 4 * s4 + 8 < 4 * N_WAVES:
                    for i in range(4 * s4 + 8, 4 * s4 + 12):
                        emit_ln_block(i)   # wave s4+2 prep
                for p in range(4):
                    if not NO_ATTN:
                        emit_attn(s4, p)
                    if s4 == N_WAVES - 1 and not NO_OUTPROJ and not NO_ATTN:
                        for i in range(4 * p, 4 * p + 4):
                            emit_outproj(i)

    nc.finalize()
    return nc


def _prep_core_inputs(x, ln_scale, ln_bias, w_qkv, b_qkv, w_out):
    """Host-side folding + per-core input maps."""
    scale = np.float32(HD ** -0.5)
    # qkv = xn@W + b_qkv, xn = z*ln_scale + ln_bias
    #   =>  z @ (ln_scale*W) + (ln_bias@W + b_qkv)
    b_eff = b_qkv + np.einsum(
        "d,dhf->hf", ln_bias.astype(np.float64), w_qkv.astype(np.float64)
    ).astype(np.float32)
    w_eff = ln_scale[:, None, None] * w_qkv
    wq = w_eff[:, :, 0:64] * scale
    wk = w_eff[:, :, 64:128]
    wv = w_eff[:, :, 128:192]
    bq = b_eff[:, 0:64] * scale
    bk = b_eff[:, 64:128]
    bv = b_eff[:, 128:192]

    in_maps = []
    for core in range(8):
        b, g = core // 2, core % 2
        hsel = slice(g * HL, (g + 1) * HL)
        # per fb=(t*4+p): [128 d-in-chunk, 8 chunks, 128 features]
        qp = wq[:, hsel].reshape(D, 4, 128)
        kp = wk[:, hsel].reshape(D, 4, 128)
        wqk = np.empty((128, 8, NCH, 128), npbf16)
        for t, w_t in enumerate((qp, kp)):
            for p in range(4):
                wqk[:, t * 4 + p] = (
                    w_t[:, p, :].reshape(NCH, 128, 128).transpose(1, 0, 2)
                ).astype(npbf16)
        wv_g = (
            wv[:, hsel].reshape(D, 512).reshape(NCH, 128, 512).transpose(1, 0, 2)
        ).astype(npbf16)
        bq_p = bq[hsel].reshape(4, 128)
        bk_p = bk[hsel].reshape(4, 128)
        bqk = np.ascontiguousarray(
            np.stack([bq_p, bk_p], axis=0).transpose(2, 0, 1)
        )
        bv1 = np.ascontiguousarray(bv[hsel].reshape(1, 512)).astype(npbf16)
        wout = (
            w_out[g * 512 : (g + 1) * 512, :].reshape(4, 128, 1024).transpose(1, 0, 2)
        ).astype(npbf16)
        in_maps.append(
            {
                "x": np.ascontiguousarray(x[b]).astype(npbf16),
                "wqk": np.ascontiguousarray(wqk),
                "wv": np.ascontiguousarray(wv_g),
                "bqk": bqk,
                "bv1": bv1,
                "vones": np.ones((1, 128), npbf16),
                "wout": np.ascontiguousarray(wout),
            }
        )
    return in_maps


def kernel(x, mask, ln_scale, ln_bias, w_qkv, b_qkv, w_out, b_out, **run_kwargs):
    x = np.asarray(x, np.float32)
    ln_scale = np.asarray(ln_scale, np.float32)
    ln_bias = np.asarray(ln_bias, np.float32)
    w_qkv = np.asarray(w_qkv, np.float32)
    b_qkv = np.asarray(b_qkv, np.float32)
    w_out = np.asarray(w_out, np.float32)
    b_out = np.asarray(b_out, np.float32)
    if "nc" not in _cache:
        _cache["nc"] = build_program()
    nc = _cache["nc"]
    in_maps = _prep_core_inputs(x, ln_scale, ln_bias, w_qkv, b_qkv, w_out)
    res = run_bass_kernel_spmd(nc, in_maps, list(range(8)), **run_kwargs)
    _cache["last_result"] = res
    out = np.empty((B, S, D), np.float32)
    for b in range(B):
        out[b] = res.results[2 * b]["out"] + res.results[2 * b + 1]["out"]
    out += np.asarray(b_out)[None, None, :]
    return out


# revision 71
# speedup vs baseline: 1.2247x; 1.0129x over previous
"""Causal self-attention block (LN -> QKV -> causal attention -> out-proj)
on 8 Trainium2 NeuronCores.

Sharding: core = 2*batch + head_group. Each core handles one batch element
(S=2048 tokens) and 8 of the 16 heads (tensor-parallel split of w_qkv along
the head axis and w_out along its input dim). The two partial outputs per
batch are summed on the host (the all-reduce of the sharding hint).

v2 kernel layout strategy (per core), all matmuls in bf16 (1 cycle/row at
any moving width, validated ~2e-3 end-to-end rel err on host):
  - LayerNorm in natural layout [s, d] (DVE stats), then PE-transpose the
    bf16 xn to xnT [d, s] per 512-token superblock.
  - QKV computes qT/kT in [head_dim, s] layout and V in natural [s, hd].
  - Scores are computed transposed, ST[k, q] = k.q (causal mask folded in
    on the PE via a NEG upper-tri addend), exp on ACT writes bf16 PT.
  - PV runs in natural layout: y[q, hd] accumulates over k-blocks with PT
    slices as the stationary operand and V (+ ones column) moving; the
    ones column yields softmax row-sums per-partition, so normalization is
    a per-partition reciprocal + scalar multiply (no cross-partition
    broadcast, no DMA round-trips).
  - Normalized y is PE-transposed back to yT [d_local, s] for the output
    projection.
  - Persistent tensors are split into per-block tiles so phases overlap
    through slice-exact dependencies.
"""

import os

_jp = os.environ.get("JAX_PLATFORMS")
if _jp and "axon" not in _jp:
    os.environ["JAX_PLATFORMS"] = f"axon,{_jp}"

import ml_dtypes
import numpy as np

import concourse.bass as bass
import concourse.mybir as mybir
import concourse.tile as tile
from concourse import bacc
from concourse.bass_utils import run_bass_kernel_spmd
from concourse.masks import make_identity

B, S, D, H, HD = 4, 2048, 1024, 16, 64
HL = H // 2          # heads per core (local)
NCH = D // 128       # 8 contraction chunks
NSB = S // 128       # 16 s-blocks
NQS = S // 512       # 4 superblocks
NEG = -1.0e38
LN_EPS = 1e-6

f32 = mybir.dt.float32
bf16 = mybir.dt.bfloat16
npbf16 = ml_dtypes.bfloat16

_cache = {}

# bisection knobs (full kernel: 4, False, False, False)
N_WAVES = 4
NO_ATTN = False
NO_EPI = False
NO_OUTPROJ = False


def build_program(with_vbias=True):
    nc = bacc.Bacc()

    x_d = nc.declare_dram_parameter("x", [S, D], bf16, isOutput=False)
    wqk_d = nc.declare_dram_parameter("wqk", [128, 8, NCH, 128], bf16, isOutput=False)
    wv_d = nc.declare_dram_parameter("wv", [128, NCH, 512], bf16, isOutput=False)
    bqk_d = nc.declare_dram_parameter("bqk", [128, 2, 4], f32, isOutput=False)
    if with_vbias:
        bv1_d = nc.declare_dram_parameter("bv1", [1, 512], bf16, isOutput=False)
        vones_d = nc.declare_dram_parameter("vones", [1, 128], bf16, isOutput=False)
    wout_d = nc.declare_dram_parameter("wout", [128, 4, 1024], bf16, isOutput=False)
    out_d = nc.declare_dram_parameter("out", [S, D], f32, isOutput=True)

    with tile.TileContext(nc, pool_alloc_mode="queue") as tc:
        with (
            tc.tile_pool(name="singles", bufs=1) as singles,
            tc.tile_pool(name="persist", bufs=1) as persist,
            tc.tile_pool(name="pscm", bufs=1, space="PSUM") as pscm,
            tc.tile_pool(name="atmp", bufs=4) as atmp,
            tc.tile_pool(name="astat", bufs=8) as astat,
            tc.tile_pool(name="ptp", bufs=3) as ptp,
            tc.tile_pool(name="cpool", bufs=2) as cpool,
            tc.tile_pool(name="ypool", bufs=3) as ypool,
        ):
            # ---- constants ----
            identb = singles.tile([128, 128], bf16)
            make_identity(nc, identb)
            maskTb = singles.tile([128, 128], bf16)
            nc.gpsimd.memset(maskTb, 0.0)
            nc.gpsimd.affine_select(
                out=maskTb, in_=maskTb,
                compare_op=mybir.AluOpType.is_ge,
                fill=NEG, base=0,
                pattern=[[1, 128]], channel_multiplier=-1,
            )
            eps_t = singles.tile([128, 1], f32)
            nc.vector.memset(eps_t, LN_EPS)

            # ---- persistent per-block tiles ----
            xnT_t = [persist.tile([128, NCH, 512], bf16, tag=f"xnT{s4}",
                                  name=f"xnT{s4}") for s4 in range(NQS)]
            qT_t = {(p, s4): persist.tile([128, 512], bf16, tag=f"qT{p}_{s4}",
                                          name=f"qT{p}_{s4}")
                    for p in range(4) for s4 in range(NQS)}
            kT_t = {(p, s4): persist.tile([128, 512], bf16, tag=f"kT{p}_{s4}",
                                          name=f"kT{p}_{s4}")
                    for p in range(4) for s4 in range(NQS)}
            vpp_t = [persist.tile([128, HL, HD + 1], bf16, tag=f"vpp{j}",
                                  name=f"vpp{j}") for j in range(NSB)]
            yT_t = {}   # filled by attention epilogues with yTq slices
            for j in range(NSB):
                nc.gpsimd.memset(vpp_t[j][:, :, HD : HD + 1], 1.0)

            # ---- weights ----
            wqk_all = persist.tile([128, 8, NCH, 128], bf16, tag="wqk")
            wqk_w = [wqk_all[:, fb] for fb in range(8)]
            wv_w = persist.tile([128, NCH, 512], bf16, tag="wv")
            wout_w = persist.tile([128, 4, 1024], bf16, tag="wout")

            # ================= Phase A: LayerNorm + transpose =================
            x_tiles = {}

            def emit_x_dma(i):
                x_t = atmp.tile([128, D], bf16, tag="x", bufs=12, name=f"x_{i}")
                nc.sync.dma_start(out=x_t, in_=x_d[i * 128 : (i + 1) * 128, :])
                x_tiles[i] = x_t

            def emit_ln_block(i):
                x_t = x_tiles.pop(i)
                stats = astat.tile([128, 2, 6], f32, tag="stats")
                nc.vector.bn_stats(out=stats[:, 0, :], in_=x_t[:, 0:512])
                nc.vector.bn_stats(out=stats[:, 1, :], in_=x_t[:, 512:1024])
                mv = astat.tile([128, 2], f32, tag="mv")
                nc.vector.bn_aggr(out=mv, in_=stats)
                std_t = astat.tile([128, 1], f32, tag="std")
                nc.scalar.activation(
                    out=std_t, in_=mv[:, 1:2],
                    func=mybir.ActivationFunctionType.Sqrt,
                    bias=eps_t, scale=1.0,
                )
                rstd_t = astat.tile([128, 1], f32, tag="rstd")
                nc.vector.reciprocal(out=rstd_t, in_=std_t)
                xn_t = atmp.tile([128, D], bf16, tag="xn", bufs=3, name=f"xn_{i}")
                if i % 4 == 1:
                    # ACT path: xn = (x - mu)*rstd = x*rstd + (-mu*rstd)
                    nb = astat.tile([128, 1], f32, tag="nb")
                    nc.vector.tensor_scalar(
                        out=nb, in0=mv[:, 0:1],
                        scalar1=rstd_t, scalar2=-1.0,
                        op0=mybir.AluOpType.mult, op1=mybir.AluOpType.mult,
                    )
                    nc.scalar.activation(
                        out=xn_t, in_=x_t,
                        func=mybir.ActivationFunctionType.Identity,
                        bias=nb, scale=rstd_t,
                    )
                else:
                    norm_eng = nc.vector if i % 4 == 3 else nc.gpsimd
                    norm_eng.tensor_scalar(
                        out=xn_t, in0=x_t,
                        scalar1=mv[:, 0:1], scalar2=rstd_t,
                        op0=mybir.AluOpType.subtract, op1=mybir.AluOpType.mult,
                    )
                s4, ib = i // 4, i % 4
                pst = pscm.tile([128, NCH, 128], bf16, tag="aux", bufs=1,
                                name=f"pst_{i}")
                for c in range(NCH):
                    nc.tensor.transpose(
                        pst[:, c, :],
                        xn_t[:, c * 128 : (c + 1) * 128],
                        identb,
                    )
                dst = xnT_t[s4][:, :, ib * 128 : (ib + 1) * 128]
                if i % 2 == 0:
                    nc.scalar.activation(
                        out=dst, in_=pst,
                        func=mybir.ActivationFunctionType.Copy,
                    )
                else:
                    nc.vector.tensor_copy(dst, pst)

            # ================= Phase B: QKV projection =================
            def emit_qk(t, p, s4):
                fb = t * 4 + p
                dest = qT_t if t == 0 else kT_t
                ps = pscm.tile([128, 512], f32,
                               tag=("qkv" if fb % 2 == 0 else "aux"), bufs=1,
                               name=f"psqk_{t}_{p}_{s4}")
                for c in range(NCH):
                    nc.tensor.matmul(
                        ps,
                        wqk_w[fb][:, c, :],
                        xnT_t[s4][:, c, :],
                        start=(c == 0),
                        stop=(c == NCH - 1),
                    )
                nc.vector.tensor_scalar_add(
                    out=dest[(p, s4)], in0=ps,
                    scalar1=bqk_t[:, t, p : p + 1],
                )

            def emit_v(i):
                s4, ib = i // 4, i % 4
                psv = pscm.tile([128, 512], f32,
                                tag=("qkv" if i % 2 == 0 else "aux"), bufs=1,
                                name=f"psv_{i}")
                for c in range(NCH):
                    nc.tensor.matmul(
                        psv,
                        xnT_t[s4][:, c, ib * 128 : (ib + 1) * 128],
                        wv_w[:, c, :],
                        start=(c == 0),
                        stop=(not with_vbias and c == NCH - 1),
                    )
                if with_vbias:
                    nc.tensor.matmul(psv, vones_t, bv1_t, start=False,
                                     stop=True)
                nc.vector.tensor_copy(
                    vpp_t[i][:, :, 0:HD],
                    psv.rearrange("p (h v) -> p h v", v=HD),
                )

            # ================= Phase C: causal attention =================
            def emit_attn(s4, p):
                q0 = s4 * 512
                jmax = 4 * s4 + 3
                ya = pscm.tile([128, 2, 4, 128], f32, tag="ya", bufs=1,
                               name=f"ya_{s4}_{p}")
                yacc = [ya[:, 0], ya[:, 1]]
                pts = {}

                def emit_scores(j):
                    r = max(0, j - 4 * s4)
                    diag = j >= 4 * s4
                    L = 512 - 128 * r
                    hb = 512
                    st = pscm.tile([128, 1024], f32, tag="st", bufs=2,
                                   name=f"st_{s4}_{p}_{j}")
                    kt = kT_t[(p, j // 4)]
                    # bank-aligned matmul outputs only (mid-bank column
                    # offsets fault the device); mask accumulates after
                    for hf in range(2):
                        rows = slice(hf * HD, (hf + 1) * HD)
                        nc.tensor.matmul(
                            st[:, hf * 512 : hf * 512 + L],
                            kt[rows, (j % 4) * 128 : (j % 4 + 1) * 128],
                            qT_t[(p, s4)][rows, r * 128 : 512],
                            start=True, stop=not diag,
                        )
                    if diag:
                        for hf in range(2):
                            nc.tensor.matmul(
                                st[:, hf * 512 : hf * 512 + 128],
                                identb, maskTb,
                                start=False, stop=True,
                            )
                    pt = ptp.tile([128, 1024], bf16, tag="pt", bufs=8,
                                  name=f"pt_{s4}_{p}_{j}")
                    if L < 512:
                        # [L, 512) was never written; exp the two written
                        # ranges separately
                        for hf in range(2):
                            nc.scalar.activation(
                                out=pt[:, hf * 512 : hf * 512 + L],
                                in_=st[:, hf * 512 : hf * 512 + L],
                                func=mybir.ActivationFunctionType.Exp,
                            )
                    else:
                        nc.scalar.activation(
                            out=pt[:, 0 : 512 + L], in_=st[:, 0 : 512 + L],
                            func=mybir.ActivationFunctionType.Exp,
                        )
                    pts[j] = (pt, r, hb)

                def emit_pv(j):
                    # PSUM has one accumulation group per 2KB bank: start
                    # only on the first matmul into each hf's bank (zeroes
                    # whole-bank has_written); later first-touches of other
                    # qb columns overwrite via per-element has_written.
                    pt, r, hb = pts.pop(j)
                    for hf in range(2):
                        head = 2 * p + hf
                        for qb in range(r, 4):
                            base = hf * hb + (qb - r) * 128
                            nc.tensor.matmul(
                                yacc[hf][:, qb, 0 : HD + 1],
                                pt[:, base : base + 128],
                                vpp_t[j][:, head, :],
                                start=(j == 0 and qb == 0),
                                stop=(j == jmax and qb == 3),
                            )

                emit_scores(0)
                for j in range(1, jmax + 1):
                    emit_scores(j)
                    emit_pv(j - 1)
                emit_pv(jmax)
                if NO_EPI:
                    return

                # epilogue: per-partition softmax normalization
                ynat = cpool.tile([128, 4, 128], bf16, tag="ynat",
                                  name=f"ynat_{s4}_{p}")
                rc = cpool.tile([128, 2, 4, 1], f32, tag="rc",
                                name=f"rc_{s4}_{p}")
                nc.vector.reciprocal(out=rc, in_=ya[:, :, :, HD : HD + 1])
                if s4 == 3 and p == 3:
                    # last unit: shortest chain — DVE reads PSUM directly
                    for qb in range(4):
                        for hf in range(2):
                            nc.vector.tensor_scalar_mul(
                                out=ynat[:, qb, hf * HD : (hf + 1) * HD],
                                in0=ya[:, hf, qb, 0:HD],
                                scalar1=rc[:, hf, qb, :],
                            )
                else:
                    yraw = cpool.tile([128, 2, 4, HD], f32, tag="yraw",
                                      name=f"yraw_{s4}_{p}")
                    nc.vector.tensor_copy(yraw, ya[:, :, :, 0:HD])
                    for qb in range(4):
                        for hf in range(2):
                            nc.gpsimd.tensor_scalar_mul(
                                out=ynat[:, qb, hf * HD : (hf + 1) * HD],
                                in0=yraw[:, hf, qb, :],
                                scalar1=rc[:, hf, qb, :],
                            )
                ytp = pscm.tile([128, 4, 128], bf16, tag="aux", bufs=1,
                                name=f"ytp_{s4}_{p}")
                for qb in range(4):
                    nc.tensor.transpose(ytp[:, qb, :], ynat[:, qb, :], identb)
                yTq = persist.tile([128, 4, 128], bf16, tag=f"yTq_{s4}_{p}",
                                   name=f"yTq_{s4}_{p}")
                nc.vector.tensor_copy(yTq, ytp)
                for qb in range(4):
                    yT_t[(4 * s4 + qb, p)] = yTq[:, qb, :]

            # ================= out-projection =================
            def emit_outproj(i):
                y_t = ypool.tile([128, 1024], f32, tag="y", name=f"y_{i}")
                for nh in range(2):
                    pso = pscm.tile([128, 512], f32,
                                    tag=("aux" if nh == 0 else "qkv"), bufs=1,
                                    name=f"pso_{i}_{nh}")
                    for c in range(4):
                        nc.tensor.matmul(
                            pso,
                            yT_t[(i, c)],
                            wout_w[:, c, nh * 512 : (nh + 1) * 512],
                            start=(c == 0),
                            stop=(c == 3),
                        )
                    nc.vector.tensor_copy(y_t[:, nh * 512 : (nh + 1) * 512], pso)
                nc.sync.dma_start(out=out_d[i * 128 : (i + 1) * 128, :], in_=y_t)

            # ---- emission schedule: software-pipelined waves ----
            # LN runs one wave ahead of QKV, which runs with attention of
            # the prior wave; outproj of wave w-1 interleaves into wave w.
            for i in range(4):
                emit_x_dma(i)
            for fb in (0, 4):
                nc.sync.dma_start(out=wqk_all[:, fb], in_=wqk_d[:, fb])
            for i in range(4, 8):
                emit_x_dma(i)
            for fb in (1, 5):
                nc.sync.dma_start(out=wqk_all[:, fb], in_=wqk_d[:, fb])
            nc.sync.dma_start(out=wv_w, in_=wv_d[:, :, :])
            for i in range(8, 12):
                emit_x_dma(i)
            for fb in (2, 6, 3, 7):
                nc.sync.dma_start(out=wqk_all[:, fb], in_=wqk_d[:, fb])
            nc.sync.dma_start(out=wout_w, in_=wout_d[:, :, :])
            bqk_t = singles.tile([128, 2, 4], f32)
            nc.sync.dma_start(out=bqk_t, in_=bqk_d[:, :, :])
            if with_vbias:
                bv1_t = singles.tile([1, 512], bf16)
                nc.sync.dma_start(out=bv1_t, in_=bv1_d[:, :])
                vones_t = singles.tile([1, 128], bf16)
                nc.sync.dma_start(out=vones_t, in_=vones_d[:, :])
            for i in range(0, min(8, 4 * N_WAVES)):
                emit_ln_block(i)        # waves 0+1
            for s4 in range(N_WAVES):
                for p in range(4):
                    emit_qk(0, p, s4)
                    emit_qk(1, p, s4)
                for i in range(4 * s4, 4 * s4 + 4):
                    emit_v(i)
                if s4 == 0:
                    for i in range(12, NSB):
                        emit_x_dma(i)
                if s4 < 2 and 4 * s4 + 8 < 4 * N_WAVES:
                    for i in range(4 * s4 + 8, 4 * s4 + 12):
                        emit_ln_block(i)   # wave s4+2 prep
                for p in range(4):
                    if not NO_ATTN:
                        emit_attn(s4, p)
                    if s4 == N_WAVES - 1 and not NO_OUTPROJ and not NO_ATTN:
                        for i in range(4 * p, 4 * p + 4):
                            emit_outproj(i)

    nc.finalize()
    return nc


def _prep_core_inputs(x, ln_scale, ln_bias, w_qkv, b_qkv, w_out,
                      with_vbias=True):
    """Host-side folding + per-core input maps."""
    scale = np.float32(HD ** -0.5)
    # qkv = xn@W + b_qkv, xn = z*ln_scale + ln_bias
    #   =>  z @ (ln_scale*W) + (ln_bias@W + b_qkv)
    b_eff = b_qkv + np.einsum(
        "d,dhf->hf", ln_bias.astype(np.float64), w_qkv.astype(np.float64)
    ).astype(np.float32)
    w_eff = ln_scale[:, None, None] * w_qkv
    wq = w_eff[:, :, 0:64] * scale
    wk = w_eff[:, :, 64:128]
    wv = w_eff[:, :, 128:192]
    bq = b_eff[:, 0:64] * scale
    bk = b_eff[:, 64:128]
    bv = b_eff[:, 128:192]

    in_maps = []
    for core in range(8):
        b, g = core // 2, core % 2
        hsel = slice(g * HL, (g + 1) * HL)
        # per fb=(t*4+p): [128 d-in-chunk, 8 chunks, 128 features]
        qp = wq[:, hsel].reshape(D, 4, 128)
        kp = wk[:, hsel].reshape(D, 4, 128)
        wqk = np.empty((128, 8, NCH, 128), npbf16)
        for t, w_t in enumerate((qp, kp)):
            for p in range(4):
                wqk[:, t * 4 + p] = (
                    w_t[:, p, :].reshape(NCH, 128, 128).transpose(1, 0, 2)
                ).astype(npbf16)
        wv_g = (
            wv[:, hsel].reshape(D, 512).reshape(NCH, 128, 512).transpose(1, 0, 2)
        ).astype(npbf16)
        bq_p = bq[hsel].reshape(4, 128)
        bk_p = bk[hsel].reshape(4, 128)
        bqk = np.ascontiguousarray(
            np.stack([bq_p, bk_p], axis=0).transpose(2, 0, 1)
        )
        bv1 = np.ascontiguousarray(bv[hsel].reshape(1, 512)).astype(npbf16) \
            if with_vbias else None
        wout = (
            w_out[g * 512 : (g + 1) * 512, :].reshape(4, 128, 1024).transpose(1, 0, 2)
        ).astype(npbf16)
        im = {
            "x": np.ascontiguousarray(x[b]).astype(npbf16),
            "wqk": np.ascontiguousarray(wqk),
            "wv": np.ascontiguousarray(wv_g),
            "bqk": bqk,
            "wout": np.ascontiguousarray(wout),
        }
        if with_vbias:
            im["bv1"] = bv1
            im["vones"] = np.ones((1, 128), npbf16)
        in_maps.append(im)
    return in_maps


def kernel(x, mask, ln_scale, ln_bias, w_qkv, b_qkv, w_out, b_out, **run_kwargs):
    x = np.asarray(x, np.float32)
    ln_scale = np.asarray(ln_scale, np.float32)
    ln_bias = np.asarray(ln_bias, np.float32)
    w_qkv = np.asarray(w_qkv, np.float32)
    b_qkv = np.asarray(b_qkv, np.float32)
    w_out = np.asarray(w_out, np.float32)
    b_out = np.asarray(b_out, np.float32)
    b_eff_v = b_qkv[:, 128:192] + np.einsum(
        "d,dhf->hf", ln_bias.astype(np.float64),
        w_qkv[:, :, 128:192].astype(np.float64)).astype(np.float32)
    with_vbias = bool(np.any(b_eff_v))
    key = ("nc", with_vbias)
    if key not in _cache:
        _cache[key] = build_program(with_vbias)
    nc = _cache[key]
    _cache["nc"] = nc
    in_maps = _prep_core_inputs(x, ln_scale, ln_bias, w_qkv, b_qkv, w_out,
                                with_vbias)
    res = run_bass_kernel_spmd(nc, in_maps, list(range(8)), **run_kwargs)
    _cache["last_result"] = res
    out = np.empty((B, S, D), np.float32)
    for b in range(B):
        out[b] = res.results[2 * b]["out"] + res.results[2 * b + 1]["out"]
    out += np.asarray(b_out)[None, None, :]
    return out


# revision 73
# speedup vs baseline: 1.2256x; 1.0007x over previous
"""Causal self-attention block (LN -> QKV -> causal attention -> out-proj)
on 8 Trainium2 NeuronCores.

Sharding: core = 2*batch + head_group. Each core handles one batch element
(S=2048 tokens) and 8 of the 16 heads (tensor-parallel split of w_qkv along
the head axis and w_out along its input dim). The two partial outputs per
batch are summed on the host (the all-reduce of the sharding hint).

v2 kernel layout strategy (per core), all matmuls in bf16 (1 cycle/row at
any moving width, validated ~2e-3 end-to-end rel err on host):
  - LayerNorm in natural layout [s, d] (DVE stats), then PE-transpose the
    bf16 xn to xnT [d, s] per 512-token superblock.
  - QKV computes qT/kT in [head_dim, s] layout and V in natural [s, hd].
  - Scores are computed transposed, ST[k, q] = k.q (causal mask folded in
    on the PE via a NEG upper-tri addend), exp on ACT writes bf16 PT.
  - PV runs in natural layout: y[q, hd] accumulates over k-blocks with PT
    slices as the stationary operand and V (+ ones column) moving; the
    ones column yields softmax row-sums per-partition, so normalization is
    a per-partition reciprocal + scalar multiply (no cross-partition
    broadcast, no DMA round-trips).
  - Normalized y is PE-transposed back to yT [d_local, s] for the output
    projection.
  - Persistent tensors are split into per-block tiles so phases overlap
    through slice-exact dependencies.
"""

import os

_jp = os.environ.get("JAX_PLATFORMS")
if _jp and "axon" not in _jp:
    os.environ["JAX_PLATFORMS"] = f"axon,{_jp}"

import ml_dtypes
import numpy as np

import concourse.bass as bass
import concourse.mybir as mybir
import concourse.tile as tile
from concourse import bacc
from concourse.bass_utils import run_bass_kernel_spmd
from concourse.masks import make_identity

B, S, D, H, HD = 4, 2048, 1024, 16, 64
HL = H // 2          # heads per core (local)
NCH = D // 128       # 8 contraction chunks
NSB = S // 128       # 16 s-blocks
NQS = S // 512       # 4 superblocks
NEG = -1.0e38
LN_EPS = 1e-6

f32 = mybir.dt.float32
bf16 = mybir.dt.bfloat16
npbf16 = ml_dtypes.bfloat16

_cache = {}

# bisection knobs (full kernel: 4, False, False, False)
N_WAVES = 4
NO_ATTN = False
NO_EPI = False
NO_OUTPROJ = False


def build_program(with_vbias=True):
    nc = bacc.Bacc()

    x_d = nc.declare_dram_parameter("x", [S, D], bf16, isOutput=False)
    wqk_d = nc.declare_dram_parameter("wqk", [128, 8, NCH, 128], bf16, isOutput=False)
    wv_d = nc.declare_dram_parameter("wv", [128, NCH, 512], bf16, isOutput=False)
    bqk_d = nc.declare_dram_parameter("bqk", [128, 2, 4], f32, isOutput=False)
    if with_vbias:
        bv1_d = nc.declare_dram_parameter("bv1", [1, 512], bf16, isOutput=False)
        vones_d = nc.declare_dram_parameter("vones", [1, 128], bf16, isOutput=False)
    wout_d = nc.declare_dram_parameter("wout", [128, 4, 1024], bf16, isOutput=False)
    out_d = nc.declare_dram_parameter("out", [S, D], f32, isOutput=True)

    with tile.TileContext(nc, pool_alloc_mode="queue") as tc:
        with (
            tc.tile_pool(name="singles", bufs=1) as singles,
            tc.tile_pool(name="persist", bufs=1) as persist,
            tc.tile_pool(name="pscm", bufs=1, space="PSUM") as pscm,
            tc.tile_pool(name="atmp", bufs=4) as atmp,
            tc.tile_pool(name="astat", bufs=8) as astat,
            tc.tile_pool(name="ptp", bufs=3) as ptp,
            tc.tile_pool(name="cpool", bufs=4) as cpool,
            tc.tile_pool(name="ypool", bufs=4) as ypool,
        ):
            # ---- constants ----
            identb = singles.tile([128, 128], bf16)
            make_identity(nc, identb)
            maskTb = singles.tile([128, 128], bf16)
            nc.gpsimd.memset(maskTb, 0.0)
            nc.gpsimd.affine_select(
                out=maskTb, in_=maskTb,
                compare_op=mybir.AluOpType.is_ge,
                fill=NEG, base=0,
                pattern=[[1, 128]], channel_multiplier=-1,
            )
            eps_t = singles.tile([128, 1], f32)
            nc.vector.memset(eps_t, LN_EPS)

            # ---- persistent per-block tiles ----
            xnT_t = [persist.tile([128, NCH, 512], bf16, tag=f"xnT{s4}",
                                  name=f"xnT{s4}") for s4 in range(NQS)]
            qT_t = {(p, s4): persist.tile([128, 512], bf16, tag=f"qT{p}_{s4}",
                                          name=f"qT{p}_{s4}")
                    for p in range(4) for s4 in range(NQS)}
            kT_t = {(p, s4): persist.tile([128, 512], bf16, tag=f"kT{p}_{s4}",
                                          name=f"kT{p}_{s4}")
                    for p in range(4) for s4 in range(NQS)}
            vpp_t = [persist.tile([128, HL, HD + 1], bf16, tag=f"vpp{j}",
                                  name=f"vpp{j}") for j in range(NSB)]
            yT_t = {}   # filled by attention epilogues with yTq slices
            for j in range(NSB):
                nc.gpsimd.memset(vpp_t[j][:, :, HD : HD + 1], 1.0)

            # ---- weights ----
            wqk_all = persist.tile([128, 8, NCH, 128], bf16, tag="wqk")
            wqk_w = [wqk_all[:, fb] for fb in range(8)]
            wv_w = persist.tile([128, NCH, 512], bf16, tag="wv")
            wout_w = persist.tile([128, 4, 1024], bf16, tag="wout")

            # ================= Phase A: LayerNorm + transpose =================
            x_tiles = {}

            def emit_x_dma(i):
                x_t = atmp.tile([128, D], bf16, tag="x", bufs=12, name=f"x_{i}")
                nc.sync.dma_start(out=x_t, in_=x_d[i * 128 : (i + 1) * 128, :])
                x_tiles[i] = x_t

            def emit_ln_block(i):
                x_t = x_tiles.pop(i)
                stats = astat.tile([128, 2, 6], f32, tag="stats")
                nc.vector.bn_stats(out=stats[:, 0, :], in_=x_t[:, 0:512])
                nc.vector.bn_stats(out=stats[:, 1, :], in_=x_t[:, 512:1024])
                mv = astat.tile([128, 2], f32, tag="mv")
                nc.vector.bn_aggr(out=mv, in_=stats)
                std_t = astat.tile([128, 1], f32, tag="std")
                nc.scalar.activation(
                    out=std_t, in_=mv[:, 1:2],
                    func=mybir.ActivationFunctionType.Sqrt,
                    bias=eps_t, scale=1.0,
                )
                rstd_t = astat.tile([128, 1], f32, tag="rstd")
                nc.vector.reciprocal(out=rstd_t, in_=std_t)
                xn_t = atmp.tile([128, D], bf16, tag="xn", bufs=3, name=f"xn_{i}")
                if i % 4 == 1:
                    # ACT path: xn = (x - mu)*rstd = x*rstd + (-mu*rstd)
                    nb = astat.tile([128, 1], f32, tag="nb")
                    nc.vector.tensor_scalar(
                        out=nb, in0=mv[:, 0:1],
                        scalar1=rstd_t, scalar2=-1.0,
                        op0=mybir.AluOpType.mult, op1=mybir.AluOpType.mult,
                    )
                    nc.scalar.activation(
                        out=xn_t, in_=x_t,
                        func=mybir.ActivationFunctionType.Identity,
                        bias=nb, scale=rstd_t,
                    )
                else:
                    norm_eng = nc.vector if i % 4 == 3 else nc.gpsimd
                    norm_eng.tensor_scalar(
                        out=xn_t, in0=x_t,
                        scalar1=mv[:, 0:1], scalar2=rstd_t,
                        op0=mybir.AluOpType.subtract, op1=mybir.AluOpType.mult,
                    )
                s4, ib = i // 4, i % 4
                pst = pscm.tile([128, NCH, 128], bf16, tag="aux", bufs=1,
                                name=f"pst_{i}")
                for c in range(NCH):
                    nc.tensor.transpose(
                        pst[:, c, :],
                        xn_t[:, c * 128 : (c + 1) * 128],
                        identb,
                    )
                dst = xnT_t[s4][:, :, ib * 128 : (ib + 1) * 128]
                if i % 2 == 0:
                    nc.scalar.activation(
                        out=dst, in_=pst,
                        func=mybir.ActivationFunctionType.Copy,
                    )
                else:
                    nc.vector.tensor_copy(dst, pst)

            # ================= Phase B: QKV projection =================
            def emit_qk(t, p, s4):
                fb = t * 4 + p
                dest = qT_t if t == 0 else kT_t
                ps = pscm.tile([128, 512], f32,
                               tag=("qkv" if fb % 2 == 0 else "aux"), bufs=1,
                               name=f"psqk_{t}_{p}_{s4}")
                for c in range(NCH):
                    nc.tensor.matmul(
                        ps,
                        wqk_w[fb][:, c, :],
                        xnT_t[s4][:, c, :],
                        start=(c == 0),
                        stop=(c == NCH - 1),
                    )
                nc.vector.tensor_scalar_add(
                    out=dest[(p, s4)], in0=ps,
                    scalar1=bqk_t[:, t, p : p + 1],
                )

            def emit_v(i):
                s4, ib = i // 4, i % 4
                psv = pscm.tile([128, 512], f32,
                                tag=("qkv" if i % 2 == 0 else "aux"), bufs=1,
                                name=f"psv_{i}")
                for c in range(NCH):
                    nc.tensor.matmul(
                        psv,
                        xnT_t[s4][:, c, ib * 128 : (ib + 1) * 128],
                        wv_w[:, c, :],
                        start=(c == 0),
                        stop=(not with_vbias and c == NCH - 1),
                    )
                if with_vbias:
                    nc.tensor.matmul(psv, vones_t, bv1_t, start=False,
                                     stop=True)
                nc.vector.tensor_copy(
                    vpp_t[i][:, :, 0:HD],
                    psv.rearrange("p (h v) -> p h v", v=HD),
                )

            # ================= Phase C: causal attention =================
            def emit_attn(s4, p):
                q0 = s4 * 512
                jmax = 4 * s4 + 3
                ya = pscm.tile([128, 2, 4, 128], f32, tag="ya", bufs=1,
                               name=f"ya_{s4}_{p}")
                yacc = [ya[:, 0], ya[:, 1]]
                pts = {}

                def emit_scores(j):
                    r = max(0, j - 4 * s4)
                    diag = j >= 4 * s4
                    L = 512 - 128 * r
                    hb = 512
                    st = pscm.tile([128, 1024], f32, tag="st", bufs=2,
                                   name=f"st_{s4}_{p}_{j}")
                    kt = kT_t[(p, j // 4)]
                    # bank-aligned matmul outputs only (mid-bank column
                    # offsets fault the device); mask accumulates after
                    for hf in range(2):
                        rows = slice(hf * HD, (hf + 1) * HD)
                        nc.tensor.matmul(
                            st[:, hf * 512 : hf * 512 + L],
                            kt[rows, (j % 4) * 128 : (j % 4 + 1) * 128],
                            qT_t[(p, s4)][rows, r * 128 : 512],
                            start=True, stop=not diag,
                        )
                    if diag:
                        for hf in range(2):
                            nc.tensor.matmul(
                                st[:, hf * 512 : hf * 512 + 128],
                                identb, maskTb,
                                start=False, stop=True,
                            )
                    pt = ptp.tile([128, 1024], bf16, tag="pt", bufs=8,
                                  name=f"pt_{s4}_{p}_{j}")
                    if L < 512:
                        # [L, 512) was never written; exp the two written
                        # ranges separately
                        for hf in range(2):
                            nc.scalar.activation(
                                out=pt[:, hf * 512 : hf * 512 + L],
                                in_=st[:, hf * 512 : hf * 512 + L],
                                func=mybir.ActivationFunctionType.Exp,
                            )
                    else:
                        nc.scalar.activation(
                            out=pt[:, 0 : 512 + L], in_=st[:, 0 : 512 + L],
                            func=mybir.ActivationFunctionType.Exp,
                        )
                    pts[j] = (pt, r, hb)

                def emit_pv(j):
                    # PSUM has one accumulation group per 2KB bank: start
                    # only on the first matmul into each hf's bank (zeroes
                    # whole-bank has_written); later first-touches of other
                    # qb columns overwrite via per-element has_written.
                    pt, r, hb = pts.pop(j)
                    for hf in range(2):
                        head = 2 * p + hf
                        for qb in range(r, 4):
                            base = hf * hb + (qb - r) * 128
                            nc.tensor.matmul(
                                yacc[hf][:, qb, 0 : HD + 1],
                                pt[:, base : base + 128],
                                vpp_t[j][:, head, :],
                                start=(j == 0 and qb == 0),
                                stop=(j == jmax and qb == 3),
                            )

                emit_scores(0)
                for j in range(1, jmax + 1):
                    emit_scores(j)
                    emit_pv(j - 1)
                emit_pv(jmax)
                if NO_EPI:
                    return

                # epilogue: per-partition softmax normalization
                ynat = cpool.tile([128, 4, 128], bf16, tag="ynat",
                                  name=f"ynat_{s4}_{p}")
                rc = cpool.tile([128, 2, 4, 1], f32, tag="rc",
                                name=f"rc_{s4}_{p}")
                nc.vector.reciprocal(out=rc, in_=ya[:, :, :, HD : HD + 1])
                if s4 == 3 and p == 3:
                    # last unit: shortest chain — DVE reads PSUM directly
                    for qb in range(4):
                        for hf in range(2):
                            nc.vector.tensor_scalar_mul(
                                out=ynat[:, qb, hf * HD : (hf + 1) * HD],
                                in0=ya[:, hf, qb, 0:HD],
                                scalar1=rc[:, hf, qb, :],
                            )
                else:
                    yraw = cpool.tile([128, 2, 4, HD], f32, tag="yraw",
                                      name=f"yraw_{s4}_{p}")
                    nc.vector.tensor_copy(yraw, ya[:, :, :, 0:HD])
                    for qb in range(4):
                        for hf in range(2):
                            nc.gpsimd.tensor_scalar_mul(
                                out=ynat[:, qb, hf * HD : (hf + 1) * HD],
                                in0=yraw[:, hf, qb, :],
                                scalar1=rc[:, hf, qb, :],
                            )
                ytp = pscm.tile([128, 4, 128], bf16, tag="aux", bufs=1,
                                name=f"ytp_{s4}_{p}")
                for qb in range(4):
                    nc.tensor.transpose(ytp[:, qb, :], ynat[:, qb, :], identb)
                yTq = persist.tile([128, 4, 128], bf16, tag=f"yTq_{s4}_{p}",
                                   name=f"yTq_{s4}_{p}")
                nc.vector.tensor_copy(yTq, ytp)
                for qb in range(4):
                    yT_t[(4 * s4 + qb, p)] = yTq[:, qb, :]

            # ================= out-projection =================
            def emit_outproj(i):
                y_t = ypool.tile([128, 1024], f32, tag="y", name=f"y_{i}")
                for nh in range(2):
                    pso = pscm.tile([128, 512], f32,
                                    tag=("aux" if nh == 0 else "qkv"), bufs=1,
                                    name=f"pso_{i}_{nh}")
                    for c in range(4):
                        nc.tensor.matmul(
                            pso,
                            yT_t[(i, c)],
                            wout_w[:, c, nh * 512 : (nh + 1) * 512],
                            start=(c == 0),
                            stop=(c == 3),
                        )
                    nc.vector.tensor_copy(y_t[:, nh * 512 : (nh + 1) * 512], pso)
                nc.sync.dma_start(out=out_d[i * 128 : (i + 1) * 128, :], in_=y_t)

            # ---- emission schedule: software-pipelined waves ----
            # LN runs one wave ahead of QKV, which runs with attention of
            # the prior wave; outproj of wave w-1 interleaves into wave w.
            for i in range(4):
                emit_x_dma(i)
            for fb in (0, 4):
                nc.sync.dma_start(out=wqk_all[:, fb], in_=wqk_d[:, fb])
            for i in range(4, 8):
                emit_x_dma(i)
            for fb in (1, 5):
                nc.sync.dma_start(out=wqk_all[:, fb], in_=wqk_d[:, fb])
            nc.sync.dma_start(out=wv_w, in_=wv_d[:, :, :])
            for i in range(8, 12):
                emit_x_dma(i)
            for fb in (2, 6, 3, 7):
                nc.sync.dma_start(out=wqk_all[:, fb], in_=wqk_d[:, fb])
            nc.sync.dma_start(out=wout_w, in_=wout_d[:, :, :])
            bqk_t = singles.tile([128, 2, 4], f32)
            nc.sync.dma_start(out=bqk_t, in_=bqk_d[:, :, :])
            if with_vbias:
                bv1_t = singles.tile([1, 512], bf16)
                nc.sync.dma_start(out=bv1_t, in_=bv1_d[:, :])
                vones_t = singles.tile([1, 128], bf16)
                nc.sync.dma_start(out=vones_t, in_=vones_d[:, :])
            for i in range(0, min(8, 4 * N_WAVES)):
                emit_ln_block(i)        # waves 0+1
            for s4 in range(N_WAVES):
                for p in range(4):
                    emit_qk(0, p, s4)
                    emit_qk(1, p, s4)
                for i in range(4 * s4, 4 * s4 + 4):
                    emit_v(i)
                if s4 == 0:
                    for i in range(12, NSB):
                        emit_x_dma(i)
                if s4 < 2 and 4 * s4 + 8 < 4 * N_WAVES:
                    for i in range(4 * s4 + 8, 4 * s4 + 12):
                        emit_ln_block(i)   # wave s4+2 prep
                for p in range(4):
                    if not NO_ATTN:
                        emit_attn(s4, p)
                    if s4 == N_WAVES - 1 and not NO_OUTPROJ and not NO_ATTN:
                        for i in range(4 * p, 4 * p + 4):
                            emit_outproj(i)

    nc.finalize()
    return nc


def _prep_core_inputs(x, ln_scale, ln_bias, w_qkv, b_qkv, w_out,
                      with_vbias=True):
    """Host-side folding + per-core input maps."""
    scale = np.float32(HD ** -0.5)
    # qkv = xn@W + b_qkv, xn = z*ln_scale + ln_bias
    #   =>  z @ (ln_scale*W) + (ln_bias@W + b_qkv)
    b_eff = b_qkv + np.einsum(
        "d,dhf->hf", ln_bias.astype(np.float64), w_qkv.astype(np.float64)
    ).astype(np.float32)
    w_eff = ln_scale[:, None, None] * w_qkv
    wq = w_eff[:, :, 0:64] * scale
    wk = w_eff[:, :, 64:128]
    wv = w_eff[:, :, 128:192]
    bq = b_eff[:, 0:64] * scale
    bk = b_eff[:, 64:128]
    bv = b_eff[:, 128:192]

    in_maps = []
    for core in range(8):
        b, g = core // 2, core % 2
        hsel = slice(g * HL, (g + 1) * HL)
        # per fb=(t*4+p): [128 d-in-chunk, 8 chunks, 128 features]
        qp = wq[:, hsel].reshape(D, 4, 128)
        kp = wk[:, hsel].reshape(D, 4, 128)
        wqk = np.empty((128, 8, NCH, 128), npbf16)
        for t, w_t in enumerate((qp, kp)):
            for p in range(4):
                wqk[:, t * 4 + p] = (
                    w_t[:, p, :].reshape(NCH, 128, 128).transpose(1, 0, 2)
                ).astype(npbf16)
        wv_g = (
            wv[:, hsel].reshape(D, 512).reshape(NCH, 128, 512).transpose(1, 0, 2)
        ).astype(npbf16)
        bq_p = bq[hsel].reshape(4, 128)
        bk_p = bk[hsel].reshape(4, 128)
        bqk = np.ascontiguousarray(
            np.stack([bq_p, bk_p], axis=0).transpose(2, 0, 1)
        )
        bv1 = np.ascontiguousarray(bv[hsel].reshape(1, 512)).astype(npbf16) \
            if with_vbias else None
        wout = (
            w_out[g * 512 : (g + 1) * 512, :].reshape(4, 128, 1024).transpose(1, 0, 2)
        ).astype(npbf16)
        im = {
            "x": np.ascontiguousarray(x[b]).astype(npbf16),
            "wqk": np.ascontiguousarray(wqk),
            "wv": np.ascontiguousarray(wv_g),
            "bqk": bqk,
            "wout": np.ascontiguousarray(wout),
        }
        if with_vbias:
            im["bv1"] = bv1
            im["vones"] = np.ones((1, 128), npbf16)
        in_maps.append(im)
    return in_maps


def kernel(x, mask, ln_scale, ln_bias, w_qkv, b_qkv, w_out, b_out, **run_kwargs):
    x = np.asarray(x, np.float32)
    ln_scale = np.asarray(ln_scale, np.float32)
    ln_bias = np.asarray(ln_bias, np.float32)
    w_qkv = np.asarray(w_qkv, np.float32)
    b_qkv = np.asarray(b_qkv, np.float32)
    w_out = np.asarray(w_out, np.float32)
    b_out = np.asarray(b_out, np.float32)
    b_eff_v = b_qkv[:, 128:192] + np.einsum(
        "d,dhf->hf", ln_bias.astype(np.float64),
        w_qkv[:, :, 128:192].astype(np.float64)).astype(np.float32)
    with_vbias = bool(np.any(b_eff_v))
    key = ("nc", with_vbias)
    if key not in _cache:
        _cache[key] = build_program(with_vbias)
    nc = _cache[key]
    _cache["nc"] = nc
    in_maps = _prep_core_inputs(x, ln_scale, ln_bias, w_qkv, b_qkv, w_out,
                                with_vbias)
    res = run_bass_kernel_spmd(nc, in_maps, list(range(8)), **run_kwargs)
    _cache["last_result"] = res
    out = np.empty((B, S, D), np.float32)
    for b in range(B):
        out[b] = res.results[2 * b]["out"] + res.results[2 * b + 1]["out"]
    out += np.asarray(b_out)[None, None, :]
    return out


# revision 74
# speedup vs baseline: 1.2334x; 1.0064x over previous
"""Causal self-attention block (LN -> QKV -> causal attention -> out-proj)
on 8 Trainium2 NeuronCores.

Sharding: core = 2*batch + head_group. Each core handles one batch element
(S=2048 tokens) and 8 of the 16 heads (tensor-parallel split of w_qkv along
the head axis and w_out along its input dim). The two partial outputs per
batch are summed on the host (the all-reduce of the sharding hint).

v2 kernel layout strategy (per core), all matmuls in bf16 (1 cycle/row at
any moving width, validated ~2e-3 end-to-end rel err on host):
  - LayerNorm in natural layout [s, d] (DVE stats), then PE-transpose the
    bf16 xn to xnT [d, s] per 512-token superblock.
  - QKV computes qT/kT in [head_dim, s] layout and V in natural [s, hd].
  - Scores are computed transposed, ST[k, q] = k.q (causal mask folded in
    on the PE via a NEG upper-tri addend), exp on ACT writes bf16 PT.
  - PV runs in natural layout: y[q, hd] accumulates over k-blocks with PT
    slices as the stationary operand and V (+ ones column) moving; the
    ones column yields softmax row-sums per-partition, so normalization is
    a per-partition reciprocal + scalar multiply (no cross-partition
    broadcast, no DMA round-trips).
  - Normalized y is PE-transposed back to yT [d_local, s] for the output
    projection.
  - Persistent tensors are split into per-block tiles so phases overlap
    through slice-exact dependencies.
"""

import os

_jp = os.environ.get("JAX_PLATFORMS")
if _jp and "axon" not in _jp:
    os.environ["JAX_PLATFORMS"] = f"axon,{_jp}"

import ml_dtypes
import numpy as np

import concourse.bass as bass
import concourse.mybir as mybir
import concourse.tile as tile
from concourse import bacc
from concourse.bass_utils import run_bass_kernel_spmd
from concourse.masks import make_identity

B, S, D, H, HD = 4, 2048, 1024, 16, 64
HL = H // 2          # heads per core (local)
NCH = D // 128       # 8 contraction chunks
NSB = S // 128       # 16 s-blocks
NQS = S // 512       # 4 superblocks
NEG = -1.0e38
LN_EPS = 1e-6

f32 = mybir.dt.float32
bf16 = mybir.dt.bfloat16
npbf16 = ml_dtypes.bfloat16

_cache = {}

# bisection knobs (full kernel: 4, False, False, False)
N_WAVES = 4
NO_ATTN = False
NO_EPI = False
NO_OUTPROJ = False


def build_program(with_vbias=True):
    nc = bacc.Bacc()

    x_d = nc.declare_dram_parameter("x", [S, D], bf16, isOutput=False)
    wqk_d = nc.declare_dram_parameter("wqk", [128, 8, NCH, 128], bf16, isOutput=False)
    wv_d = nc.declare_dram_parameter("wv", [128, NCH, 512], bf16, isOutput=False)
    bqk_d = nc.declare_dram_parameter("bqk", [128, 2, 4], f32, isOutput=False)
    if with_vbias:
        bv1_d = nc.declare_dram_parameter("bv1", [1, 512], bf16, isOutput=False)
        vones_d = nc.declare_dram_parameter("vones", [1, 128], bf16, isOutput=False)
    wout_d = nc.declare_dram_parameter("wout", [128, 4, 1024], bf16, isOutput=False)
    out_d = nc.declare_dram_parameter("out", [S, D], f32, isOutput=True)

    with tile.TileContext(nc, pool_alloc_mode="queue") as tc:
        with (
            tc.tile_pool(name="singles", bufs=1) as singles,
            tc.tile_pool(name="persist", bufs=1) as persist,
            tc.tile_pool(name="pscm", bufs=1, space="PSUM") as pscm,
            tc.tile_pool(name="atmp", bufs=4) as atmp,
            tc.tile_pool(name="astat", bufs=8) as astat,
            tc.tile_pool(name="ptp", bufs=3) as ptp,
            tc.tile_pool(name="cpool", bufs=4) as cpool,
            tc.tile_pool(name="ypool", bufs=4) as ypool,
        ):
            # ---- constants ----
            identb = singles.tile([128, 128], bf16)
            make_identity(nc, identb)
            maskTb = singles.tile([128, 128], bf16)
            nc.gpsimd.memset(maskTb, 0.0)
            nc.gpsimd.affine_select(
                out=maskTb, in_=maskTb,
                compare_op=mybir.AluOpType.is_ge,
                fill=NEG, base=0,
                pattern=[[1, 128]], channel_multiplier=-1,
            )
            eps_t = singles.tile([128, 1], f32)
            nc.vector.memset(eps_t, LN_EPS)

            # ---- persistent per-block tiles ----
            xnT_t = [persist.tile([128, NCH, 512], bf16, tag=f"xnT{s4}",
                                  name=f"xnT{s4}") for s4 in range(NQS)]
            qT_t = {(p, s4): persist.tile([128, 512], bf16, tag=f"qT{p}_{s4}",
                                          name=f"qT{p}_{s4}")
                    for p in range(4) for s4 in range(NQS)}
            kT_t = {(p, s4): persist.tile([128, 512], bf16, tag=f"kT{p}_{s4}",
                                          name=f"kT{p}_{s4}")
                    for p in range(4) for s4 in range(NQS)}
            vpp_t = [persist.tile([128, HL, HD + 1], bf16, tag=f"vpp{j}",
                                  name=f"vpp{j}") for j in range(NSB)]
            yT_t = {}   # filled by attention epilogues with yTq slices
            for j in range(NSB):
                nc.gpsimd.memset(vpp_t[j][:, :, HD : HD + 1], 1.0)

            # ---- weights ----
            wqk_all = persist.tile([128, 8, NCH, 128], bf16, tag="wqk")
            wqk_w = [wqk_all[:, fb] for fb in range(8)]
            wv_w = persist.tile([128, NCH, 512], bf16, tag="wv")
            wout_w = persist.tile([128, 4, 1024], bf16, tag="wout")

            # ================= Phase A: LayerNorm + transpose =================
            x_tiles = {}

            def emit_x_dma(i):
                x_t = atmp.tile([128, D], bf16, tag="x", bufs=12, name=f"x_{i}")
                nc.sync.dma_start(out=x_t, in_=x_d[i * 128 : (i + 1) * 128, :])
                x_tiles[i] = x_t

            def emit_ln_block(i):
                x_t = x_tiles.pop(i)
                stats = astat.tile([128, 2, 6], f32, tag="stats")
                nc.vector.bn_stats(out=stats[:, 0, :], in_=x_t[:, 0:512])
                nc.vector.bn_stats(out=stats[:, 1, :], in_=x_t[:, 512:1024])
                mv = astat.tile([128, 2], f32, tag="mv")
                nc.vector.bn_aggr(out=mv, in_=stats)
                std_t = astat.tile([128, 1], f32, tag="std")
                nc.scalar.activation(
                    out=std_t, in_=mv[:, 1:2],
                    func=mybir.ActivationFunctionType.Sqrt,
                    bias=eps_t, scale=1.0,
                )
                rstd_t = astat.tile([128, 1], f32, tag="rstd")
                nc.vector.reciprocal(out=rstd_t, in_=std_t)
                xn_t = atmp.tile([128, D], bf16, tag="xn", bufs=3, name=f"xn_{i}")
                if i % 4 == 1:
                    # ACT path: xn = (x - mu)*rstd = x*rstd + (-mu*rstd)
                    nb = astat.tile([128, 1], f32, tag="nb")
                    nc.vector.tensor_scalar(
                        out=nb, in0=mv[:, 0:1],
                        scalar1=rstd_t, scalar2=-1.0,
                        op0=mybir.AluOpType.mult, op1=mybir.AluOpType.mult,
                    )
                    nc.scalar.activation(
                        out=xn_t, in_=x_t,
                        func=mybir.ActivationFunctionType.Identity,
                        bias=nb, scale=rstd_t,
                    )
                else:
                    norm_eng = nc.vector if i % 4 == 3 else nc.gpsimd
                    norm_eng.tensor_scalar(
                        out=xn_t, in0=x_t,
                        scalar1=mv[:, 0:1], scalar2=rstd_t,
                        op0=mybir.AluOpType.subtract, op1=mybir.AluOpType.mult,
                    )
                s4, ib = i // 4, i % 4
                pst = pscm.tile([128, NCH, 128], bf16, tag="aux", bufs=1,
                                name=f"pst_{i}")
                for c in range(NCH):
                    nc.tensor.transpose(
                        pst[:, c, :],
                        xn_t[:, c * 128 : (c + 1) * 128],
                        identb,
                    )
                dst = xnT_t[s4][:, :, ib * 128 : (ib + 1) * 128]
                if i % 2 == 0:
                    nc.scalar.activation(
                        out=dst, in_=pst,
                        func=mybir.ActivationFunctionType.Copy,
                    )
                else:
                    nc.vector.tensor_copy(dst, pst)

            # ================= Phase B: QKV projection =================
            def emit_qk(t, p, s4):
                fb = t * 4 + p
                dest = qT_t if t == 0 else kT_t
                ps = pscm.tile([128, 512], f32,
                               tag=("qkv" if fb % 2 == 0 else "aux"), bufs=1,
                               name=f"psqk_{t}_{p}_{s4}")
                for c in range(NCH):
                    nc.tensor.matmul(
                        ps,
                        wqk_w[fb][:, c, :],
                        xnT_t[s4][:, c, :],
                        start=(c == 0),
                        stop=(c == NCH - 1),
                    )
                nc.vector.tensor_scalar_add(
                    out=dest[(p, s4)], in0=ps,
                    scalar1=bqk_t[:, t, p : p + 1],
                )

            def emit_v(i):
                s4, ib = i // 4, i % 4
                psv = pscm.tile([128, 512], f32,
                                tag=("qkv" if i % 2 == 0 else "aux"), bufs=1,
                                name=f"psv_{i}")
                for c in range(NCH):
                    nc.tensor.matmul(
                        psv,
                        xnT_t[s4][:, c, ib * 128 : (ib + 1) * 128],
                        wv_w[:, c, :],
                        start=(c == 0),
                        stop=(not with_vbias and c == NCH - 1),
                    )
                if with_vbias:
                    nc.tensor.matmul(psv, vones_t, bv1_t, start=False,
                                     stop=True)
                nc.vector.tensor_copy(
                    vpp_t[i][:, :, 0:HD],
                    psv.rearrange("p (h v) -> p h v", v=HD),
                )

            # ================= Phase C: causal attention =================
            def emit_attn(s4, p):
                q0 = s4 * 512
                jmax = 4 * s4 + 3
                ya = pscm.tile([128, 2, 4, 128], f32, tag="ya", bufs=1,
                               name=f"ya_{s4}_{p}")
                yacc = [ya[:, 0], ya[:, 1]]
                pts = {}

                def emit_scores(j):
                    r = max(0, j - 4 * s4)
                    diag = j >= 4 * s4
                    L = 512 - 128 * r
                    hb = 512
                    st = pscm.tile([128, 1024], f32, tag="st", bufs=2,
                                   name=f"st_{s4}_{p}_{j}")
                    kt = kT_t[(p, j // 4)]
                    # bank-aligned matmul outputs only (mid-bank column
                    # offsets fault the device); mask accumulates after
                    for hf in range(2):
                        rows = slice(hf * HD, (hf + 1) * HD)
                        nc.tensor.matmul(
                            st[:, hf * 512 : hf * 512 + L],
                            kt[rows, (j % 4) * 128 : (j % 4 + 1) * 128],
                            qT_t[(p, s4)][rows, r * 128 : 512],
                            start=True, stop=not diag,
                        )
                    if diag:
                        if L < 512:
                            # K=128 filler fills hf0's [L, 512) strip so a
                            # single wide exp reads only written PSUM (the
                            # strip's pt output is never consumed)
                            nc.tensor.matmul(
                                st[:, L:512],
                                identb,
                                qT_t[(p, s4)][:, 0 : 512 - L],
                                start=False, stop=False,
                            )
                        for hf in range(2):
                            nc.tensor.matmul(
                                st[:, hf * 512 : hf * 512 + 128],
                                identb, maskTb,
                                start=False, stop=True,
                            )
                    pt = ptp.tile([128, 1024], bf16, tag="pt", bufs=8,
                                  name=f"pt_{s4}_{p}_{j}")
                    nc.scalar.activation(
                        out=pt[:, 0 : 512 + L], in_=st[:, 0 : 512 + L],
                        func=mybir.ActivationFunctionType.Exp,
                    )
                    pts[j] = (pt, r, hb)

                def emit_pv(j):
                    # PSUM has one accumulation group per 2KB bank: start
                    # only on the first matmul into each hf's bank (zeroes
                    # whole-bank has_written); later first-touches of other
                    # qb columns overwrite via per-element has_written.
                    pt, r, hb = pts.pop(j)
                    for hf in range(2):
                        head = 2 * p + hf
                        for qb in range(r, 4):
                            base = hf * hb + (qb - r) * 128
                            nc.tensor.matmul(
                                yacc[hf][:, qb, 0 : HD + 1],
                                pt[:, base : base + 128],
                                vpp_t[j][:, head, :],
                                start=(j == 0 and qb == 0),
                                stop=(j == jmax and qb == 3),
                            )

                emit_scores(0)
                for j in range(1, jmax + 1):
                    emit_scores(j)
                    emit_pv(j - 1)
                emit_pv(jmax)
                if NO_EPI:
                    return

                # epilogue: per-partition softmax normalization
                ynat = cpool.tile([128, 4, 128], bf16, tag="ynat",
                                  name=f"ynat_{s4}_{p}")
                rc = cpool.tile([128, 2, 4, 1], f32, tag="rc",
                                name=f"rc_{s4}_{p}")
                nc.vector.reciprocal(out=rc, in_=ya[:, :, :, HD : HD + 1])
                if s4 == 3 and p == 3:
                    # last unit: shortest chain — DVE reads PSUM directly
                    for qb in range(4):
                        for hf in range(2):
                            nc.vector.tensor_scalar_mul(
                                out=ynat[:, qb, hf * HD : (hf + 1) * HD],
                                in0=ya[:, hf, qb, 0:HD],
                                scalar1=rc[:, hf, qb, :],
                            )
                else:
                    yraw = cpool.tile([128, 2, 4, HD], f32, tag="yraw",
                                      name=f"yraw_{s4}_{p}")
                    nc.vector.tensor_copy(yraw, ya[:, :, :, 0:HD])
                    for qb in range(4):
                        for hf in range(2):
                            nc.gpsimd.tensor_scalar_mul(
                                out=ynat[:, qb, hf * HD : (hf + 1) * HD],
                                in0=yraw[:, hf, qb, :],
                                scalar1=rc[:, hf, qb, :],
                            )
                ytp = pscm.tile([128, 4, 128], bf16, tag="aux", bufs=1,
                                name=f"ytp_{s4}_{p}")
                for qb in range(4):
                    nc.tensor.transpose(ytp[:, qb, :], ynat[:, qb, :], identb)
                yTq = persist.tile([128, 4, 128], bf16, tag=f"yTq_{s4}_{p}",
                                   name=f"yTq_{s4}_{p}")
                nc.vector.tensor_copy(yTq, ytp)
                for qb in range(4):
                    yT_t[(4 * s4 + qb, p)] = yTq[:, qb, :]

            # ================= out-projection =================
            def emit_outproj(i):
                y_t = ypool.tile([128, 1024], f32, tag="y", name=f"y_{i}")
                for nh in range(2):
                    pso = pscm.tile([128, 512], f32,
                                    tag=("aux" if nh == 0 else "qkv"), bufs=1,
                                    name=f"pso_{i}_{nh}")
                    for c in range(4):
                        nc.tensor.matmul(
                            pso,
                            yT_t[(i, c)],
                            wout_w[:, c, nh * 512 : (nh + 1) * 512],
                            start=(c == 0),
                            stop=(c == 3),
                        )
                    nc.vector.tensor_copy(y_t[:, nh * 512 : (nh + 1) * 512], pso)
                nc.sync.dma_start(out=out_d[i * 128 : (i + 1) * 128, :], in_=y_t)

            # ---- emission schedule: software-pipelined waves ----
            # LN runs one wave ahead of QKV, which runs with attention of
            # the prior wave; outproj of wave w-1 interleaves into wave w.
            for i in range(4):
                emit_x_dma(i)
            for fb in (0, 4):
                nc.sync.dma_start(out=wqk_all[:, fb], in_=wqk_d[:, fb])
            for i in range(4, 8):
                emit_x_dma(i)
            for fb in (1, 5):
                nc.sync.dma_start(out=wqk_all[:, fb], in_=wqk_d[:, fb])
            nc.sync.dma_start(out=wv_w, in_=wv_d[:, :, :])
            for i in range(8, 12):
                emit_x_dma(i)
            for fb in (2, 6, 3, 7):
                nc.sync.dma_start(out=wqk_all[:, fb], in_=wqk_d[:, fb])
            nc.sync.dma_start(out=wout_w, in_=wout_d[:, :, :])
            bqk_t = singles.tile([128, 2, 4], f32)
            nc.sync.dma_start(out=bqk_t, in_=bqk_d[:, :, :])
            if with_vbias:
                bv1_t = singles.tile([1, 512], bf16)
                nc.sync.dma_start(out=bv1_t, in_=bv1_d[:, :])
                vones_t = singles.tile([1, 128], bf16)
                nc.sync.dma_start(out=vones_t, in_=vones_d[:, :])
            for i in range(0, min(8, 4 * N_WAVES)):
                emit_ln_block(i)        # waves 0+1
            for s4 in range(N_WAVES):
                for p in range(4):
                    emit_qk(0, p, s4)
                    emit_qk(1, p, s4)
                for i in range(4 * s4, 4 * s4 + 4):
                    emit_v(i)
                if s4 == 0:
                    for i in range(12, NSB):
                        emit_x_dma(i)
                if s4 < 2 and 4 * s4 + 8 < 4 * N_WAVES:
                    for i in range(4 * s4 + 8, 4 * s4 + 12):
                        emit_ln_block(i)   # wave s4+2 prep
                for p in range(4):
                    if not NO_ATTN:
                        emit_attn(s4, p)
                    if s4 == N_WAVES - 1 and not NO_OUTPROJ and not NO_ATTN:
                        for i in range(4 * p, 4 * p + 4):
                            emit_outproj(i)

    nc.finalize()
    return nc


def _prep_core_inputs(x, ln_scale, ln_bias, w_qkv, b_qkv, w_out,
                      with_vbias=True):
    """Host-side folding + per-core input maps."""
    scale = np.float32(HD ** -0.5)
    # qkv = xn@W + b_qkv, xn = z*ln_scale + ln_bias
    #   =>  z @ (ln_scale*W) + (ln_bias@W + b_qkv)
    b_eff = b_qkv + np.einsum(
        "d,dhf->hf", ln_bias.astype(np.float64), w_qkv.astype(np.float64)
    ).astype(np.float32)
    w_eff = ln_scale[:, None, None] * w_qkv
    wq = w_eff[:, :, 0:64] * scale
    wk = w_eff[:, :, 64:128]
    wv = w_eff[:, :, 128:192]
    bq = b_eff[:, 0:64] * scale
    bk = b_eff[:, 64:128]
    bv = b_eff[:, 128:192]

    in_maps = []
    for core in range(8):
        b, g = core // 2, core % 2
        hsel = slice(g * HL, (g + 1) * HL)
        # per fb=(t*4+p): [128 d-in-chunk, 8 chunks, 128 features]
        qp = wq[:, hsel].reshape(D, 4, 128)
        kp = wk[:, hsel].reshape(D, 4, 128)
        wqk = np.empty((128, 8, NCH, 128), npbf16)
        for t, w_t in enumerate((qp, kp)):
            for p in range(4):
                wqk[:, t * 4 + p] = (
                    w_t[:, p, :].reshape(NCH, 128, 128).transpose(1, 0, 2)
                ).astype(npbf16)
        wv_g = (
            wv[:, hsel].reshape(D, 512).reshape(NCH, 128, 512).transpose(1, 0, 2)
        ).astype(npbf16)
        bq_p = bq[hsel].reshape(4, 128)
        bk_p = bk[hsel].reshape(4, 128)
        bqk = np.ascontiguousarray(
            np.stack([bq_p, bk_p], axis=0).transpose(2, 0, 1)
        )
        bv1 = np.ascontiguousarray(bv[hsel].reshape(1, 512)).astype(npbf16) \
            if with_vbias else None
        wout = (
            w_out[g * 512 : (g + 1) * 512, :].reshape(4, 128, 1024).transpose(1, 0, 2)
        ).astype(npbf16)
        im = {
            "x": np.ascontiguousarray(x[b]).astype(npbf16),
            "wqk": np.ascontiguousarray(wqk),
            "wv": np.ascontiguousarray(wv_g),
            "bqk": bqk,
            "wout": np.ascontiguousarray(wout),
        }
        if with_vbias:
            im["bv1"] = bv1
            im["vones"] = np.ones((1, 128), npbf16)
        in_maps.append(im)
    return in_maps


def kernel(x, mask, ln_scale, ln_bias, w_qkv, b_qkv, w_out, b_out, **run_kwargs):
    x = np.asarray(x, np.float32)
    ln_scale = np.asarray(ln_scale, np.float32)
    ln_bias = np.asarray(ln_bias, np.float32)
    w_qkv = np.asarray(w_qkv, np.float32)
    b_qkv = np.asarray(b_qkv, np.float32)
    w_out = np.asarray(w_out, np.float32)
    b_out = np.asarray(b_out, np.float32)
    b_eff_v = b_qkv[:, 128:192] + np.einsum(
        "d,dhf->hf", ln_bias.astype(np.float64),
        w_qkv[:, :, 128:192].astype(np.float64)).astype(np.float32)
    with_vbias = bool(np.any(b_eff_v))
    key = ("nc", with_vbias)
    if key not in _cache:
        _cache[key] = build_program(with_vbias)
    nc = _cache[key]
    _cache["nc"] = nc
    in_maps = _prep_core_inputs(x, ln_scale, ln_bias, w_qkv, b_qkv, w_out,
                                with_vbias)
    res = run_bass_kernel_spmd(nc, in_maps, list(range(8)), **run_kwargs)
    _cache["last_result"] = res
    out = np.empty((B, S, D), np.float32)
    for b in range(B):
        out[b] = res.results[2 * b]["out"] + res.results[2 * b + 1]["out"]
    out += np.asarray(b_out)[None, None, :]
    return out


# revision 75
# speedup vs baseline: 1.2402x; 1.0055x over previous
"""Causal self-attention block (LN -> QKV -> causal attention -> out-proj)
on 8 Trainium2 NeuronCores.

Sharding: core = 2*batch + head_group. Each core handles one batch element
(S=2048 tokens) and 8 of the 16 heads (tensor-parallel split of w_qkv along
the head axis and w_out along its input dim). The two partial outputs per
batch are summed on the host (the all-reduce of the sharding hint).

v2 kernel layout strategy (per core), all matmuls in bf16 (1 cycle/row at
any moving width, validated ~2e-3 end-to-end rel err on host):
  - LayerNorm in natural layout [s, d] (DVE stats), then PE-transpose the
    bf16 xn to xnT [d, s] per 512-token superblock.
  - QKV computes qT/kT in [head_dim, s] layout and V in natural [s, hd].
  - Scores are computed transposed, ST[k, q] = k.q (causal mask folded in
    on the PE via a NEG upper-tri addend), exp on ACT writes bf16 PT.
  - PV runs in natural layout: y[q, hd] accumulates over k-blocks with PT
    slices as the stationary operand and V (+ ones column) moving; the
    ones column yields softmax row-sums per-partition, so normalization is
    a per-partition reciprocal + scalar multiply (no cross-partition
    broadcast, no DMA round-trips).
  - Normalized y is PE-transposed back to yT [d_local, s] for the output
    projection.
  - Persistent tensors are split into per-block tiles so phases overlap
    through slice-exact dependencies.
"""

import os

_jp = os.environ.get("JAX_PLATFORMS")
if _jp and "axon" not in _jp:
    os.environ["JAX_PLATFORMS"] = f"axon,{_jp}"

import ml_dtypes
import numpy as np

import concourse.bass as bass
import concourse.mybir as mybir
import concourse.tile as tile
from concourse import bacc
from concourse.bass_utils import run_bass_kernel_spmd
from concourse.masks import make_identity

B, S, D, H, HD = 4, 2048, 1024, 16, 64
HL = H // 2          # heads per core (local)
NCH = D // 128       # 8 contraction chunks
NSB = S // 128       # 16 s-blocks
NQS = S // 512       # 4 superblocks
NEG = -1.0e38
LN_EPS = 1e-6

f32 = mybir.dt.float32
bf16 = mybir.dt.bfloat16
npbf16 = ml_dtypes.bfloat16

_cache = {}

# bisection knobs (full kernel: 4, False, False, False)
N_WAVES = 4
NO_ATTN = False
NO_EPI = False
NO_OUTPROJ = False


def build_program(with_vbias=True):
    nc = bacc.Bacc()

    x_d = nc.declare_dram_parameter("x", [S, D], bf16, isOutput=False)
    wqk_d = nc.declare_dram_parameter("wqk", [128, 8, NCH, 128], bf16, isOutput=False)
    wv_d = nc.declare_dram_parameter("wv", [128, NCH, 512], bf16, isOutput=False)
    bqk_d = nc.declare_dram_parameter("bqk", [128, 2, 4], f32, isOutput=False)
    if with_vbias:
        bv1_d = nc.declare_dram_parameter("bv1", [1, 512], bf16, isOutput=False)
        vones_d = nc.declare_dram_parameter("vones", [1, 128], bf16, isOutput=False)
    wout_d = nc.declare_dram_parameter("wout", [128, 4, 1024], bf16, isOutput=False)
    out_d = nc.declare_dram_parameter("out", [S, D], f32, isOutput=True)

    with tile.TileContext(nc, pool_alloc_mode="queue") as tc:
        with (
            tc.tile_pool(name="singles", bufs=1) as singles,
            tc.tile_pool(name="persist", bufs=1) as persist,
            tc.tile_pool(name="pscm", bufs=1, space="PSUM") as pscm,
            tc.tile_pool(name="atmp", bufs=4) as atmp,
            tc.tile_pool(name="astat", bufs=8) as astat,
            tc.tile_pool(name="ptp", bufs=3) as ptp,
            tc.tile_pool(name="cpool", bufs=4) as cpool,
            tc.tile_pool(name="ypool", bufs=4) as ypool,
        ):
            # ---- constants ----
            identb = singles.tile([128, 128], bf16)
            make_identity(nc, identb)
            maskTb = singles.tile([128, 128], bf16)
            nc.gpsimd.memset(maskTb, 0.0)
            nc.gpsimd.affine_select(
                out=maskTb, in_=maskTb,
                compare_op=mybir.AluOpType.is_ge,
                fill=NEG, base=0,
                pattern=[[1, 128]], channel_multiplier=-1,
            )
            eps_t = singles.tile([128, 1], f32)
            nc.vector.memset(eps_t, LN_EPS)

            # ---- persistent per-block tiles ----
            xnT_t = [persist.tile([128, NCH, 512], bf16, tag=f"xnT{s4}",
                                  name=f"xnT{s4}") for s4 in range(NQS)]
            qT_t = {(p, s4): persist.tile([128, 512], bf16, tag=f"qT{p}_{s4}",
                                          name=f"qT{p}_{s4}")
                    for p in range(4) for s4 in range(NQS)}
            kT_t = {(p, s4): persist.tile([128, 512], bf16, tag=f"kT{p}_{s4}",
                                          name=f"kT{p}_{s4}")
                    for p in range(4) for s4 in range(NQS)}
            vpp_t = [persist.tile([128, HL, HD + 1], bf16, tag=f"vpp{j}",
                                  name=f"vpp{j}") for j in range(NSB)]
            yT_t = {}   # filled by attention epilogues with yTq slices
            for j in range(NSB):
                nc.gpsimd.memset(vpp_t[j][:, :, HD : HD + 1], 1.0)

            # ---- weights ----
            wqk_all = persist.tile([128, 8, NCH, 128], bf16, tag="wqk")
            wqk_w = [wqk_all[:, fb] for fb in range(8)]
            wv_w = persist.tile([128, NCH, 512], bf16, tag="wv")
            wout_w = persist.tile([128, 4, 1024], bf16, tag="wout")

            # ================= Phase A: LayerNorm + transpose =================
            x_tiles = {}

            def emit_x_dma(i):
                x_t = atmp.tile([128, D], bf16, tag="x", bufs=12, name=f"x_{i}")
                nc.sync.dma_start(out=x_t, in_=x_d[i * 128 : (i + 1) * 128, :])
                x_tiles[i] = x_t

            def emit_ln_block(i):
                x_t = x_tiles.pop(i)
                stats = astat.tile([128, 2, 6], f32, tag="stats")
                nc.vector.bn_stats(out=stats[:, 0, :], in_=x_t[:, 0:512])
                nc.vector.bn_stats(out=stats[:, 1, :], in_=x_t[:, 512:1024])
                mv = astat.tile([128, 2], f32, tag="mv")
                nc.vector.bn_aggr(out=mv, in_=stats)
                std_t = astat.tile([128, 1], f32, tag="std")
                nc.scalar.activation(
                    out=std_t, in_=mv[:, 1:2],
                    func=mybir.ActivationFunctionType.Sqrt,
                    bias=eps_t, scale=1.0,
                )
                rstd_t = astat.tile([128, 1], f32, tag="rstd")
                nc.vector.reciprocal(out=rstd_t, in_=std_t)
                xn_t = atmp.tile([128, D], bf16, tag="xn", bufs=3, name=f"xn_{i}")
                if i % 4 == 1:
                    # ACT path: xn = (x - mu)*rstd = x*rstd + (-mu*rstd)
                    nb = astat.tile([128, 1], f32, tag="nb")
                    nc.vector.tensor_scalar(
                        out=nb, in0=mv[:, 0:1],
                        scalar1=rstd_t, scalar2=-1.0,
                        op0=mybir.AluOpType.mult, op1=mybir.AluOpType.mult,
                    )
                    nc.scalar.activation(
                        out=xn_t, in_=x_t,
                        func=mybir.ActivationFunctionType.Identity,
                        bias=nb, scale=rstd_t,
                    )
                else:
                    norm_eng = nc.vector if (i % 4 == 3 or i == 0) else nc.gpsimd
                    norm_eng.tensor_scalar(
                        out=xn_t, in0=x_t,
                        scalar1=mv[:, 0:1], scalar2=rstd_t,
                        op0=mybir.AluOpType.subtract, op1=mybir.AluOpType.mult,
                    )
                s4, ib = i // 4, i % 4
                pst = pscm.tile([128, NCH, 128], bf16, tag="aux", bufs=1,
                                name=f"pst_{i}")
                for c in range(NCH):
                    nc.tensor.transpose(
                        pst[:, c, :],
                        xn_t[:, c * 128 : (c + 1) * 128],
                        identb,
                    )
                dst = xnT_t[s4][:, :, ib * 128 : (ib + 1) * 128]
                if i % 2 == 0:
                    nc.scalar.activation(
                        out=dst, in_=pst,
                        func=mybir.ActivationFunctionType.Copy,
                    )
                else:
                    nc.vector.tensor_copy(dst, pst)

            # ================= Phase B: QKV projection =================
            def emit_qk(t, p, s4):
                fb = t * 4 + p
                dest = qT_t if t == 0 else kT_t
                ps = pscm.tile([128, 512], f32,
                               tag=("qkv" if fb % 2 == 0 else "aux"), bufs=1,
                               name=f"psqk_{t}_{p}_{s4}")
                for c in range(NCH):
                    nc.tensor.matmul(
                        ps,
                        wqk_w[fb][:, c, :],
                        xnT_t[s4][:, c, :],
                        start=(c == 0),
                        stop=(c == NCH - 1),
                    )
                nc.vector.tensor_scalar_add(
                    out=dest[(p, s4)], in0=ps,
                    scalar1=bqk_t[:, t, p : p + 1],
                )

            def emit_v(i):
                s4, ib = i // 4, i % 4
                psv = pscm.tile([128, 512], f32,
                                tag=("qkv" if i % 2 == 0 else "aux"), bufs=1,
                                name=f"psv_{i}")
                for c in range(NCH):
                    nc.tensor.matmul(
                        psv,
                        xnT_t[s4][:, c, ib * 128 : (ib + 1) * 128],
                        wv_w[:, c, :],
                        start=(c == 0),
                        stop=(not with_vbias and c == NCH - 1),
                    )
                if with_vbias:
                    nc.tensor.matmul(psv, vones_t, bv1_t, start=False,
                                     stop=True)
                nc.vector.tensor_copy(
                    vpp_t[i][:, :, 0:HD],
                    psv.rearrange("p (h v) -> p h v", v=HD),
                )

            # ================= Phase C: causal attention =================
            def emit_attn(s4, p):
                q0 = s4 * 512
                jmax = 4 * s4 + 3
                ya = pscm.tile([128, 2, 4, 128], f32, tag="ya", bufs=1,
                               name=f"ya_{s4}_{p}")
                yacc = [ya[:, 0], ya[:, 1]]
                pts = {}

                def emit_scores(j):
                    r = max(0, j - 4 * s4)
                    diag = j >= 4 * s4
                    L = 512 - 128 * r
                    hb = 512
                    st = pscm.tile([128, 1024], f32, tag="st", bufs=2,
                                   name=f"st_{s4}_{p}_{j}")
                    kt = kT_t[(p, j // 4)]
                    # bank-aligned matmul outputs only (mid-bank column
                    # offsets fault the device); mask accumulates after
                    for hf in range(2):
                        rows = slice(hf * HD, (hf + 1) * HD)
                        nc.tensor.matmul(
                            st[:, hf * 512 : hf * 512 + L],
                            kt[rows, (j % 4) * 128 : (j % 4 + 1) * 128],
                            qT_t[(p, s4)][rows, r * 128 : 512],
                            start=True, stop=not diag,
                        )
                    if diag:
                        if L < 512:
                            # K=128 filler fills hf0's [L, 512) strip so a
                            # single wide exp reads only written PSUM (the
                            # strip's pt output is never consumed)
                            nc.tensor.matmul(
                                st[:, L:512],
                                identb,
                                qT_t[(p, s4)][:, 0 : 512 - L],
                                start=False, stop=False,
                            )
                        for hf in range(2):
                            nc.tensor.matmul(
                                st[:, hf * 512 : hf * 512 + 128],
                                identb, maskTb,
                                start=False, stop=True,
                            )
                    pt = ptp.tile([128, 1024], bf16, tag="pt", bufs=8,
                                  name=f"pt_{s4}_{p}_{j}")
                    nc.scalar.activation(
                        out=pt[:, 0 : 512 + L], in_=st[:, 0 : 512 + L],
                        func=mybir.ActivationFunctionType.Exp,
                    )
                    pts[j] = (pt, r, hb)

                def emit_pv(j):
                    # PSUM has one accumulation group per 2KB bank: start
                    # only on the first matmul into each hf's bank (zeroes
                    # whole-bank has_written); later first-touches of other
                    # qb columns overwrite via per-element has_written.
                    pt, r, hb = pts.pop(j)
                    for hf in range(2):
                        head = 2 * p + hf
                        for qb in range(r, 4):
                            base = hf * hb + (qb - r) * 128
                            nc.tensor.matmul(
                                yacc[hf][:, qb, 0 : HD + 1],
                                pt[:, base : base + 128],
                                vpp_t[j][:, head, :],
                                start=(j == 0 and qb == 0),
                                stop=(j == jmax and qb == 3),
                            )

                emit_scores(0)
                for j in range(1, jmax + 1):
                    emit_scores(j)
                    emit_pv(j - 1)
                emit_pv(jmax)
                if NO_EPI:
                    return

                # epilogue: per-partition softmax normalization
                ynat = cpool.tile([128, 4, 128], bf16, tag="ynat",
                                  name=f"ynat_{s4}_{p}")
                rc = cpool.tile([128, 2, 4, 1], f32, tag="rc",
                                name=f"rc_{s4}_{p}")
                nc.vector.reciprocal(out=rc, in_=ya[:, :, :, HD : HD + 1])
                if s4 == 3 and p == 3:
                    # last unit: shortest chain — DVE reads PSUM directly
                    for qb in range(4):
                        for hf in range(2):
                            nc.vector.tensor_scalar_mul(
                                out=ynat[:, qb, hf * HD : (hf + 1) * HD],
                                in0=ya[:, hf, qb, 0:HD],
                                scalar1=rc[:, hf, qb, :],
                            )
                else:
                    yraw = cpool.tile([128, 2, 4, HD], f32, tag="yraw",
                                      name=f"yraw_{s4}_{p}")
                    nc.vector.tensor_copy(yraw, ya[:, :, :, 0:HD])
                    for qb in range(4):
                        for hf in range(2):
                            nc.gpsimd.tensor_scalar_mul(
                                out=ynat[:, qb, hf * HD : (hf + 1) * HD],
                                in0=yraw[:, hf, qb, :],
                                scalar1=rc[:, hf, qb, :],
                            )
                last_unit = s4 == 3 and p == 3
                ytp = pscm.tile([128, 4, 128], bf16,
                                tag=("st" if last_unit else "aux"),
                                bufs=(2 if last_unit else 1),
                                name=f"ytp_{s4}_{p}")
                for qb in range(4):
                    nc.tensor.transpose(ytp[:, qb, :], ynat[:, qb, :], identb)
                yTq = persist.tile([128, 4, 128], bf16, tag=f"yTq_{s4}_{p}",
                                   name=f"yTq_{s4}_{p}")
                nc.vector.tensor_copy(yTq, ytp)
                for qb in range(4):
                    yT_t[(4 * s4 + qb, p)] = yTq[:, qb, :]

            # ================= out-projection =================
            def emit_outproj(i):
                y_t = ypool.tile([128, 1024], f32, tag="y", name=f"y_{i}")
                for nh in range(2):
                    pso = pscm.tile([128, 512], f32,
                                    tag=("aux" if nh == 0 else "qkv"), bufs=1,
                                    name=f"pso_{i}_{nh}")
                    for c in range(4):
                        nc.tensor.matmul(
                            pso,
                            yT_t[(i, c)],
                            wout_w[:, c, nh * 512 : (nh + 1) * 512],
                            start=(c == 0),
                            stop=(c == 3),
                        )
                    nc.vector.tensor_copy(y_t[:, nh * 512 : (nh + 1) * 512], pso)
                    nc.sync.dma_start(
                        out=out_d[i * 128 : (i + 1) * 128,
                                  nh * 512 : (nh + 1) * 512],
                        in_=y_t[:, nh * 512 : (nh + 1) * 512],
                    )

            # ---- emission schedule: software-pipelined waves ----
            # LN runs one wave ahead of QKV, which runs with attention of
            # the prior wave; outproj of wave w-1 interleaves into wave w.
            for i in range(4):
                emit_x_dma(i)
            for fb in (0, 4):
                nc.sync.dma_start(out=wqk_all[:, fb], in_=wqk_d[:, fb])
            for i in range(4, 8):
                emit_x_dma(i)
            for fb in (1, 5):
                nc.sync.dma_start(out=wqk_all[:, fb], in_=wqk_d[:, fb])
            nc.sync.dma_start(out=wv_w, in_=wv_d[:, :, :])
            for i in range(8, 12):
                emit_x_dma(i)
            for fb in (2, 6, 3, 7):
                nc.sync.dma_start(out=wqk_all[:, fb], in_=wqk_d[:, fb])
            nc.sync.dma_start(out=wout_w, in_=wout_d[:, :, :])
            bqk_t = singles.tile([128, 2, 4], f32)
            nc.sync.dma_start(out=bqk_t, in_=bqk_d[:, :, :])
            if with_vbias:
                bv1_t = singles.tile([1, 512], bf16)
                nc.sync.dma_start(out=bv1_t, in_=bv1_d[:, :])
                vones_t = singles.tile([1, 128], bf16)
                nc.sync.dma_start(out=vones_t, in_=vones_d[:, :])
            for i in range(0, min(8, 4 * N_WAVES)):
                emit_ln_block(i)        # waves 0+1
            for s4 in range(N_WAVES):
                for p in range(4):
                    emit_qk(0, p, s4)
                    emit_qk(1, p, s4)
                for i in range(4 * s4, 4 * s4 + 4):
                    emit_v(i)
                if s4 == 0:
                    for i in range(12, NSB):
                        emit_x_dma(i)
                if s4 < 2 and 4 * s4 + 8 < 4 * N_WAVES:
                    for i in range(4 * s4 + 8, 4 * s4 + 12):
                        emit_ln_block(i)   # wave s4+2 prep
                for p in range(4):
                    if not NO_ATTN:
                        emit_attn(s4, p)
                    if s4 == N_WAVES - 1 and not NO_OUTPROJ and not NO_ATTN:
                        for i in range(4 * p, 4 * p + 4):
                            emit_outproj(i)

    nc.finalize()
    return nc


def _prep_core_inputs(x, ln_scale, ln_bias, w_qkv, b_qkv, w_out,
                      with_vbias=True):
    """Host-side folding + per-core input maps."""
    scale = np.float32(HD ** -0.5)
    # qkv = xn@W + b_qkv, xn = z*ln_scale + ln_bias
    #   =>  z @ (ln_scale*W) + (ln_bias@W + b_qkv)
    b_eff = b_qkv + np.einsum(
        "d,dhf->hf", ln_bias.astype(np.float64), w_qkv.astype(np.float64)
    ).astype(np.float32)
    w_eff = ln_scale[:, None, None] * w_qkv
    wq = w_eff[:, :, 0:64] * scale
    wk = w_eff[:, :, 64:128]
    wv = w_eff[:, :, 128:192]
    bq = b_eff[:, 0:64] * scale
    bk = b_eff[:, 64:128]
    bv = b_eff[:, 128:192]

    in_maps = []
    for core in range(8):
        b, g = core // 2, core % 2
        hsel = slice(g * HL, (g + 1) * HL)
        # per fb=(t*4+p): [128 d-in-chunk, 8 chunks, 128 features]
        qp = wq[:, hsel].reshape(D, 4, 128)
        kp = wk[:, hsel].reshape(D, 4, 128)
        wqk = np.empty((128, 8, NCH, 128), npbf16)
        for t, w_t in enumerate((qp, kp)):
            for p in range(4):
                wqk[:, t * 4 + p] = (
                    w_t[:, p, :].reshape(NCH, 128, 128).transpose(1, 0, 2)
                ).astype(npbf16)
        wv_g = (
            wv[:, hsel].reshape(D, 512).reshape(NCH, 128, 512).transpose(1, 0, 2)
        ).astype(npbf16)
        bq_p = bq[hsel].reshape(4, 128)
        bk_p = bk[hsel].reshape(4, 128)
        bqk = np.ascontiguousarray(
            np.stack([bq_p, bk_p], axis=0).transpose(2, 0, 1)
        )
        bv1 = np.ascontiguousarray(bv[hsel].reshape(1, 512)).astype(npbf16) \
            if with_vbias else None
        wout = (
            w_out[g * 512 : (g + 1) * 512, :].reshape(4, 128, 1024).transpose(1, 0, 2)
        ).astype(npbf16)
        im = {
            "x": np.ascontiguousarray(x[b]).astype(npbf16),
            "wqk": np.ascontiguousarray(wqk),
            "wv": np.ascontiguousarray(wv_g),
            "bqk": bqk,
            "wout": np.ascontiguousarray(wout),
        }
        if with_vbias:
            im["bv1"] = bv1
            im["vones"] = np.ones((1, 128), npbf16)
        in_maps.append(im)
    return in_maps


def kernel(x, mask, ln_scale, ln_bias, w_qkv, b_qkv, w_out, b_out, **run_kwargs):
    x = np.asarray(x, np.float32)
    ln_scale = np.asarray(ln_scale, np.float32)
    ln_bias = np.asarray(ln_bias, np.float32)
    w_qkv = np.asarray(w_qkv, np.float32)
    b_qkv = np.asarray(b_qkv, np.float32)
    w_out = np.asarray(w_out, np.float32)
    b_out = np.asarray(b_out, np.float32)
    b_eff_v = b_qkv[:, 128:192] + np.einsum(
        "d,dhf->hf", ln_bias.astype(np.float64),
        w_qkv[:, :, 128:192].astype(np.float64)).astype(np.float32)
    with_vbias = bool(np.any(b_eff_v))
    key = ("nc", with_vbias)
    if key not in _cache:
        _cache[key] = build_program(with_vbias)
    nc = _cache[key]
    _cache["nc"] = nc
    in_maps = _prep_core_inputs(x, ln_scale, ln_bias, w_qkv, b_qkv, w_out,
                                with_vbias)
    res = run_bass_kernel_spmd(nc, in_maps, list(range(8)), **run_kwargs)
    _cache["last_result"] = res
    out = np.empty((B, S, D), np.float32)
    for b in range(B):
        out[b] = res.results[2 * b]["out"] + res.results[2 * b + 1]["out"]
    out += np.asarray(b_out)[None, None, :]
    return out
